# revision 1
# baseline (speedup 1.0000x reference)
"""Trainium2 Bass kernel for nn_DEC_26139170963600 (vq_codebook).

Reference computation:
  4x strided conv1d (stride 2, VALID) with LeakyReLU(0.1) between layers,
  flatten -> soft VQ assignment over 64 centers:
      d2 = ||z||^2 + ||c||^2 - 2 z.c
      q  = (1/(1+d2)) row-normalized            (alpha=1 -> exponent is 1)

Sharding: data-parallel over batch N=256 across 8 cores (32 samples/core).
Weights / centers replicated. No cross-device communication.

Per-core kernel design (fp8 DoubleRow conv stack):
  - x and all conv weights quantized host-side to fp8e4 (TRN E4M3, max 240).
    Weights are pre-scaled by a per-layer power of two (up to ~224 max mag)
    to clear e4m3 subnormals; the inverse scale rides the PSUM eviction.
  - conv layer = K/2 tap-pair matmuls in MatmulPerfMode.DoubleRow, which
    virtualizes the PE to a 256-deep contraction (2 fp8 weights/cell):
        out[o, l] += sum_i W[o,:,k+i]^T . h[:, 2l+k+i],  i in {0,1}
    lhsT = (128, 2, 128) tap-pair slice of the packed weights; rhs is the
    contiguous slice h[k : k+2*Lout] rearranged "(l two) -> two l" (the
    stride-2 conv makes tap pairs adjacent in memory). Odd K padded with a
    zero tap (conv1 15->16, conv3 7->8; h2 rows are stride-248-padded so
    the zero tap's read stays in-bounds).
  - PSUM eviction splits LeakyReLU over three engines so no engine exceeds
    the PE's busy time:  ACT: A = ps*2^-k + b (PSUM->SBUF bf16);
    Pool (conv1) / DVE (conv2-3): t = 0.1*A;  DVE: h' = max(A, t) -> fp8
    (exact lrelu since 0.1 < 1). Two G-sample groups share a 2-bank PSUM
    tile so each eviction pass covers both in one instruction. Conv blocks
    are emitted in a four-stream interleaved order so eviction latency
    hides under another stream's PE work.
  - CAUTION (probe-verified): ACT Prelu computes exact alpha*x and would
    fuse the whole eviction into one op (sims 56 us), but it hard-faults
    the device when >=4 cores run it concurrently. Same for moving conv2's
    tensor_scalar onto Pool. Both are disabled; see EVICT_PRELU.
  - conv4 evicts bf16 z (no lrelu); distance stays bf16: 59 matmuls
    accumulate -2 z.c into PSUM (32n x 64j); ||z||^2 via per-group DVE
    square+reduce overlapped with conv4, then an fp32 matmul against a
    ones column; 1 + ||c||^2 is a host-precomputed (32,64) fp32 tile.
  - q = reciprocal(1+d2) row-normalized on DVE, DMA out as fp32.
  - PE pre-warm: dummy matmuls during the w1/x DMA lead-in so HAM
    un-throttles (1.2 -> 2.4 GHz) before real conv work arrives.

Measured (8 axon trn2 cores): max rel err 2.34e-3 vs fp32 reference;
TimelineSim 63.2 us (baseline bf16 kernel: 168.4 us). fp16 matmuls
hard-fault the device (NRT_EXEC_UNIT_UNRECOVERABLE) - do not use fp16.
"""

import os
import sys

import numpy as np
import ml_dtypes

for _p in ("/opt/trn_rl_repo",):
    if _p not in sys.path and os.path.isdir(_p):
        sys.path.insert(0, _p)

import concourse.bacc as bacc  # noqa: E402
import concourse.mybir as mybir  # noqa: E402
import concourse.tile as tile  # noqa: E402
from concourse import bass_utils  # noqa: E402

F8 = mybir.dt.float8e4
HDT = mybir.dt.bfloat16
F32 = mybir.dt.float32
AF = mybir.ActivationFunctionType
OP = mybir.AluOpType
DR = mybir.MatmulPerfMode.DoubleRow

N_CORES = 8
NS = 32          # samples per core
C = 128          # channels
KCENT = 64       # number of centers
LFIN = 59        # final length
D = C * LFIN     # 7552

# (K_real, K_padded, L_in_row_stride, L_out, L_out_row_stride, G samples/mm)
# conv2's output rows carry 1 pad element (248) so conv3's zero tap 7 reads
# in-bounds; the pad is memset once.
CFG = [
    (15, 16, 1024, 505, 505, 1),
    (12, 12, 505, 247, 248, 2),
    (7, 8, 248, 121, 121, 4),
    (4, 4, 121, 59, 59, 8),
]

N_WARM = 44  # PE pre-warm dummy matmuls

INTERLEAVE = True  # four-stream schedule vs plain layer order
# Single-pass ACT Prelu eviction sims ~4us faster but hard-faults the
# device when >=4 cores run concurrently (NRT unrecoverable; 1-2 cores
# fine, probe-verified correct). Keep the 3-op max form on hardware.
EVICT_PRELU = False


def _schedule():
    """Block emission order. Interleaved: four 8-sample conv streams s0-s3
    (conv1 blocks = 2 samples, conv2 = 4, conv3/4 = 8) and two 16-sample
    distance regions; each stage's eviction latency hides under another
    stream's PE work."""
    if not INTERLEAVE:
        return (
            [("c", 0, p) for p in range(16)]
            + [("c", 1, p) for p in range(8)]
            + [("c", 2, p) for p in range(4)]
            + [("c", 3, p) for p in range(4)]
            + [("d", 0), ("q", 0), ("d", 1), ("q", 1)]
        )
    return [
        ("c", 0, 0), ("c", 0, 1), ("c", 0, 2), ("c", 0, 3),
        ("c", 0, 4), ("c", 0, 5), ("c", 0, 6), ("c", 0, 7),
        ("c", 1, 0), ("c", 1, 1),
        ("c", 0, 8), ("c", 0, 9), ("c", 0, 10), ("c", 0, 11),
        ("c", 1, 2), ("c", 1, 3),
        ("c", 2, 0),
        ("c", 0, 12), ("c", 0, 13), ("c", 0, 14), ("c", 0, 15),
        ("c", 1, 4), ("c", 1, 5),
        ("c", 3, 0),
        ("c", 2, 1),
        ("c", 1, 6), ("c", 1, 7),
        ("c", 3, 1),
        ("c", 2, 2),
        ("d", 0),
        ("c", 2, 3),
        ("c", 3, 2),
        ("q", 0),
        ("c", 3, 3),
        ("d", 1),
        ("q", 1),
    ]

_BUILt = {}


def _build_program(n_repeat=1):
    """Build + compile the per-core Bass program (same program on all cores).

    n_repeat > 1 unrolls the full per-inference body that many times inside
    one NEFF (constants loaded once) — used only for slope timing in bench.
    """
    nc = bacc.Bacc("TRN2", target_bir_lowering=False, debug=False)

    # ---- DRAM I/O ----
    x_d = nc.dram_tensor("x", (C, NS, 1024), F8, kind="ExternalInput")
    w_d = [
        nc.dram_tensor(f"w{i+1}", (C, CFG[i][1] * C), F8, kind="ExternalInput")
        for i in range(4)
    ]
    # bias/scale pack: cols 0-3 = b1..b4; cols 4-7 = 2^-k1..2^-k4;
    # cols 8-71 = ones (zn mm rhs); cols 72-135 = row0-only 1+||c||^2
    # (cn mm rhs); cols 136-167 = e0 (partition-0 ones, cn mm lhsT)
    bp_d = nc.dram_tensor("bp", (C, 168), F32, kind="ExternalInput")
    # fp8 centers: cr8[c, l*64+j] = (-2/16)*centers[j, c*59+l], chunk 59 = 0
    cr_d = nc.dram_tensor("cr", (C, 60 * KCENT), F8, kind="ExternalInput")
    q_d = nc.dram_tensor("q", (NS, KCENT), F32, kind="ExternalOutput")

    with tile.TileContext(nc) as tc:
        with (
            tc.tile_pool(name="consts", bufs=1) as cpool,
            tc.tile_pool(name="xp", bufs=8) as xpool,
            tc.tile_pool(name="hp", bufs=1) as hpool,
            tc.tile_pool(name="ap", bufs=4) as apool,
            tc.tile_pool(name="tp", bufs=4) as tpool,
            tc.tile_pool(name="small", bufs=1) as mpool,
            tc.tile_pool(name="psA", bufs=3, space="PSUM") as psA,
            tc.tile_pool(name="psD", bufs=2, space="PSUM") as psD,
        ):
            wt = [
                cpool.tile([C, CFG[i][1] * C], F8, tag=f"w{i}", name=f"wt{i}")
                for i in range(4)
            ]
            bp = cpool.tile([C, 168], F32, tag="bp")
            cr = cpool.tile([C, 60 * KCENT], F8, tag="cr")

            for _rep in range(n_repeat):
                _body_once(nc, tc, x_d, q_d, w_d, bp_d, cr_d, wt, bp,
                           cr, xpool, hpool, apool, tpool, mpool,
                           psA, psD, load_consts=(_rep == 0))

    nc.compile()
    return nc


def _body_once(nc, tc, x_d, q_d, w_d, bp_d, cr_d, wt, bp, cr,
               xpool, hpool, apool, tpool, mpool, psA, psD,
               load_consts=True):
            # ---- Two HWDGE rings: x chunks stream on the SP ring while all
            # constants go on the ACT ring, so w1 arrives concurrently with
            # x0 and conv1 starts early ----
            if load_consts:
                nc.scalar.dma_start(wt[0][:], w_d[0].ap())
                nc.scalar.dma_start(bp[:], bp_d.ap())
            xch = []
            for g in range(16):
                t = xpool.tile([C, 2 * 1024], F8, tag="x", name=f"xch{g}")
                src = x_d.ap()[:, 2 * g : 2 * g + 2, :].rearrange("p a b -> p (a b)")
                nc.sync.dma_start(t[:], src)
                xch.append(t)
            if load_consts:
                for i in range(1, 4):
                    nc.scalar.dma_start(wt[i][:], w_d[i].ap())
                nc.scalar.dma_start(cr[:], cr_d.ap())

                # ---- PE pre-warm: HAM un-throttles (1.2 -> 2.4 GHz) after
                # ~3us of sustained activity; burn the w1/x0 DMA lead-in on
                # dummy matmuls over a zeroed scratch so conv1 starts warm ----
                wsrc = tpool.tile([1, 128], HDT, tag="warm", name="warm")
                nc.gpsimd.memset(wsrc[:], 0.0)
                wps = psA.tile([C, 128], F32, tag="ps", name="warmps")
                for _w in range(N_WARM):
                    nc.tensor.matmul(
                        wps[:], wsrc[:], wsrc[:],
                        start=(_w == 0), stop=(_w == N_WARM - 1)
                    )

            # ---- shared tiles for the conv stack / distance tail ----
            h_tiles = []
            for li, (K, Kp, Lin, Lout, Lrow, G) in enumerate(CFG):
                if li == 3:
                    hdst = hpool.tile([C, NS * Lout], HDT, tag=f"h{li}")
                else:
                    hdst = hpool.tile([C, NS * Lrow], F8, tag=f"h{li}")
                    if Lrow > Lout:
                        # zero the per-sample pad so the zero tap's
                        # in-bounds read never multiplies NaN garbage
                        padv = hdst[:].rearrange("p (n l) -> p n l", n=NS)
                        nc.gpsimd.memset(padv[:, :, Lout:Lrow], 0.0)
                h_tiles.append(hdst)
            zsq = hpool.tile([C, NS * LFIN], F32, tag="zsq", name="zsq")
            part = mpool.tile([C, NS], F32, tag="part", name="part")
            z8 = hpool.tile([C, 60 * NS], F8, tag="z8", name="z8")
            nc.gpsimd.memset(z8[:, LFIN * NS : 60 * NS], 0.0)
            z84 = z8[:].rearrange("p (l n) -> p l n", n=NS)
            cr3 = cr[:].rearrange("p (l j) -> p l j", j=KCENT)
            dps = [None] * 2

            def conv_block(li, pr):
                """One PSUM block: conv1-3 = two G-sample halves sharing a
                2-bank tile; conv4 = one 8-sample group on half a tile."""
                K, Kp, Lin, Lout, Lrow, G = CFG[li]
                hdst = h_tiles[li]
                hdst3 = hdst[:].rearrange("p (n l) -> p n l", n=NS)
                if li > 0:
                    hsrc3 = h_tiles[li - 1][:].rearrange("p (n l) -> p n l", n=NS)
                nhalf = 1 if li == 3 else 2
                gp = pr * nhalf * G
                ps = psA.tile([C, 1024], F32, tag="ps")
                for half in range(nhalf):
                    g0 = gp + half * G
                    pslice = ps[:, half * 512 : half * 512 + G * Lout]
                    for kp in range(0, Kp, 2):
                        lhsT = wt[li][:, kp * C : (kp + 2) * C].rearrange(
                            "p (two o) -> p two o", two=2
                        )
                        if li == 0:
                            x3 = xch[g0 // 2][:].rearrange("p (a b) -> p a b", a=2)
                            rhs = x3[
                                :, g0 % 2 : g0 % 2 + 1, kp : kp + 2 * Lout
                            ].rearrange("p n (l two) -> p two n l", two=2)
                        else:
                            rhs = hsrc3[
                                :, g0 : g0 + G, kp : kp + 2 * Lout
                            ].rearrange("p n (l two) -> p two n l", two=2)
                        nc.tensor.matmul(
                            pslice, lhsT, rhs,
                            start=(kp == 0), stop=(kp == Kp - 2),
                            perf_mode=DR,
                        )
                bias = bp[:, li : li + 1]
                scale = bp[:, 4 + li : 5 + li]
                ng = nhalf * G
                E = ng * Lout
                psv = (
                    ps[:, 0 : G * Lout]
                    if li == 3
                    else ps[:].rearrange("p (g l) -> p g l", g=2)[:, :, 0 : G * Lout]
                )
                if li < 3:
                    dsl = hdst3[:, gp : gp + ng, 0:Lout]
                    if EVICT_PRELU:
                        # LeakyReLU in ONE ACT pass straight to fp8:
                        # h = Prelu(ps*2^-k + b, alpha=0.1). (HW Prelu
                        # honors alpha exactly, unlike Lrelu whose table
                        # hardcodes 0.01 - probe-verified.)
                        nc.scalar.activation(
                            dsl, psv, AF.Prelu, bias=bias, scale=scale,
                            alpha=0.1,
                        )
                    else:
                        # 3-op fallback: ACT: A = ps*2^-k + b;
                        # Pool/DVE: t = 0.1A; DVE: h = max(A, t) -> fp8
                        A = apool.tile([C, E], HDT, tag="A")
                        nc.scalar.activation(
                            A[:], psv, AF.Identity, bias=bias, scale=scale
                        )
                        t = tpool.tile([C, E], HDT, tag="t")
                        ts_eng = nc.gpsimd if li < 1 else nc.vector
                        ts_eng.tensor_scalar_mul(t[:], A[:], 0.1)
                        A3 = A[:].rearrange("p (n l) -> p n l", n=ng)
                        t3 = t[:].rearrange("p (n l) -> p n l", n=ng)
                        nc.vector.tensor_tensor(dsl, A3, t3, op=OP.max)
                else:
                    # conv4: bf16 z eviction + ||z||^2 partials + fp8 z8
                    # cast (position-major, x16) for the DR distance.
                    # Engines spread to keep DVE off the tail critical path:
                    # ACT squares, Pool reduces, DVE only casts z8.
                    dsl = hdst[:, gp * Lout : (gp + ng) * Lout]
                    if EVICT_PRELU:
                        # Prelu(alpha=1) == Identity; keeps every ACT op on
                        # the Prelu table (no mid-kernel table switches)
                        nc.scalar.activation(
                            dsl, psv, AF.Prelu, bias=bias, scale=scale,
                            alpha=1.0,
                        )
                    else:
                        nc.scalar.activation(
                            dsl, psv, AF.Identity, bias=bias, scale=scale
                        )
                    zsl = zsq[:, gp * LFIN : (gp + ng) * LFIN]
                    nc.vector.tensor_tensor(zsl, dsl, dsl, op=OP.mult)
                    nc.vector.tensor_reduce(
                        part[:, gp : gp + ng],
                        zsl.rearrange("p (n l) -> p n l", n=ng),
                        axis=mybir.AxisListType.X,
                        op=OP.add,
                    )
                    z83 = z8[:].rearrange("p (l n) -> p l n", n=NS)
                    dsl3 = dsl.rearrange("p (n l) -> p n l", n=ng)
                    nc.vector.tensor_scalar_mul(
                        z83[:, 0:LFIN, gp : gp + ng].rearrange("p l n -> p n l"),
                        dsl3,
                        16.0,
                    )

            def dist_block(p):
                """d2 for 16 samples in one PSUM bank (partition base 0):
                cn (start) -> 30 fp8-DR position-pair chunks -> zn (stop);
                cn/zn are fp32 matmuls (e0 x cnrow, part x ones)."""
                dp = psD.tile([16, KCENT], F32, tag="d")
                dps[p] = dp
                nc.tensor.matmul(
                    dp[:], bp[:, 136:152], bp[:, 72:136],
                    start=True, stop=False,
                )
                for lp in range(0, 60, 2):
                    lhsT = z84[:, lp : lp + 2, 16 * p : 16 * p + 16]
                    nc.tensor.matmul(
                        dp[:], lhsT, cr3[:, lp : lp + 2, :],
                        start=False, stop=False, perf_mode=DR,
                    )
                nc.tensor.matmul(
                    dp[:], part[:, 16 * p : 16 * p + 16], bp[:, 8:72],
                    start=False, stop=True,
                )

            def q_block(p):
                """q = normalize(1/d2') for 16 samples; DMA out on the Pool
                ring (keeps the SP ring's serial dispatch off the tail)."""
                dp = dps[p]
                qn = mpool.tile([16, KCENT], F32, tag=f"qn{p}")
                nc.vector.reciprocal(qn[:], dp[:])
                rs = mpool.tile([16, 1], F32, tag=f"rs{p}")
                nc.vector.tensor_reduce(
                    rs[:], qn[:], axis=mybir.AxisListType.X, op=OP.add
                )
                rr = mpool.tile([16, 1], F32, tag=f"rr{p}")
                nc.vector.reciprocal(rr[:], rs[:])
                nc.vector.tensor_scalar_mul(qn[:], qn[:], rr[:])
                nc.sync.dma_start(q_d.ap()[16 * p : 16 * p + 16, :], qn[:])

            # ---- interleaved schedule: four 8-sample conv streams s0-s3
            # (conv1 blocks = 2 samples, conv2 = 4, conv3/4 = 8) and two
            # 16-sample distance regions; each stage's eviction latency
            # hides under another stream's PE work ----
            for blk in _schedule():
                if blk[0] == "c":
                    conv_block(blk[1], blk[2])
                elif blk[0] == "d":
                    dist_block(blk[1])
                else:
                    q_block(blk[1])


def _get_program(n_repeat=1):
    if n_repeat not in _BUILt:
        _BUILt[n_repeat] = _build_program(n_repeat)
    return _BUILt[n_repeat]


def _to_f8(a):
    """fp32 -> TRN E4M3 (max 240; clip so OCP e4m3fn bit patterns match)."""
    return np.clip(a, -240.0, 240.0).astype(ml_dtypes.float8_e4m3fn)


def _prep_inputs(x, w1, b1, w2, b2, w3, b3, w4, b4, centers):
    """Host-side prep: fp8 quantization, weight transposes, sharding."""
    ws = [w1, w2, w3, w4]
    bs = [b1, b2, b3, b4]

    const_map = {}
    scales = []
    for i, w in enumerate(ws):
        K, Kp = CFG[i][0], CFG[i][1]
        wf = np.asarray(w, np.float32)  # (O, I, K)
        # per-layer power-of-2 scale-up to ~224 max magnitude (e4m3 headroom)
        mx = float(np.abs(wf).max())
        k = int(np.floor(np.log2(224.0 / mx))) if mx > 0 else 0
        scales.append(2.0 ** (-k))
        wq = wf * (2.0 ** k)
        # (O, I, K) -> (I, Kp, O): lhsT tap k = [:, k*128:(k+1)*128]
        wp = np.zeros((C, Kp, C), np.float32)
        wp[:, :K, :] = wq.transpose(1, 2, 0)
        const_map[f"w{i+1}"] = _to_f8(wp.reshape(C, Kp * C))

    cent = np.asarray(centers, np.float32)
    # cr8[c, l*64 + j] = (-2/16) * centers[j, c*59 + l]; position chunk 59
    # is zero (pairs the z8 pad so the DR distance contracts 60 positions).
    # The 1/16 undoes z8's x16 pre-scale (both powers of 2, exact).
    cr8 = np.zeros((C, 60, KCENT), np.float32)
    cr8[:, :LFIN, :] = (
        (-2.0 / 16.0 * cent).reshape(KCENT, C, LFIN).transpose(1, 2, 0)
    )
    const_map["cr"] = _to_f8(cr8.reshape(C, 60 * KCENT))
    cn = 1.0 + (cent.astype(np.float64) ** 2).sum(axis=1)  # (64,)

    bpk = np.zeros((C, 168), np.float32)
    for i, b in enumerate(bs):
        bpk[:, i] = np.asarray(b, np.float32)
        bpk[:, 4 + i] = scales[i]
    bpk[:, 8:72] = 1.0                      # zn mm rhs (ones)
    bpk[0, 72:136] = cn.astype(np.float32)  # cn mm rhs (row 0 only)
    bpk[0, 136:168] = 1.0                   # cn mm lhsT e0 (row 0 only)
    const_map["bp"] = bpk

    xf = np.asarray(x, np.float32)
    in_maps = []
    for c in range(N_CORES):
        shard = xf[c * NS : (c + 1) * NS]  # (32, 128, 1024)
        xc = _to_f8(np.ascontiguousarray(shard.transpose(1, 0, 2)))  # (128,32,1024)
        in_maps.append({"x": xc, **const_map})
    return in_maps


def _ensure_devices():
    """Absorb wedged-device attach faults with a tiny op before the real run.

    A previous process can leave a NeuronCore wedged
    (NRT_EXEC_UNIT_UNRECOVERABLE); the first attach after a wedge fails and
    triggers a reset that completes within ~60 s.
    """
    import time

    import jax
    import jax.numpy as jnp

    for attempt in range(3):
        try:
            outs = [jax.device_put(jnp.zeros((8,)), d) + 1.0 for d in jax.devices()]
            jax.block_until_ready(outs)
            return
        except Exception:  # noqa: BLE001 - device fault; wait out the reset
            if attempt == 2:
                raise
            time.sleep(60)


def run(trace=False, **inputs):
    """Run the kernel; returns (q_full, BassKernelResults).

    Retries on device-unrecoverable faults (see _ensure_devices).
    """
    import time

    _ensure_devices()
    nc = _get_program()
    in_maps = _prep_inputs(**inputs)
    last_err = None
    for attempt in range(3):
        try:
            res = bass_utils.run_bass_kernel_spmd(
                nc, in_maps, core_ids=list(range(N_CORES)), trace=trace
            )
            break
        except Exception as e:  # noqa: BLE001 - device fault, wait + retry
            last_err = e
            if "UNAVAILABLE" not in str(e) and "unrecoverable" not in str(e).lower():
                raise
            time.sleep(60)
    else:
        raise last_err
    q = np.concatenate([res.results[c]["q"] for c in range(N_CORES)], axis=0)
    return np.ascontiguousarray(q.astype(np.float32)), res


def kernel(**inputs) -> np.ndarray:
    q, _ = run(trace=False, **inputs)
    return q



# revision 2
# speedup vs baseline: 1.0197x; 1.0197x over previous
"""Trainium2 Bass kernel for nn_DEC_26139170963600 (vq_codebook).

Reference computation:
  4x strided conv1d (stride 2, VALID) with LeakyReLU(0.1) between layers,
  flatten -> soft VQ assignment over 64 centers:
      d2 = ||z||^2 + ||c||^2 - 2 z.c
      q  = (1/(1+d2)) row-normalized            (alpha=1 -> exponent is 1)

Sharding: data-parallel over batch N=256 across 8 cores (32 samples/core).
Weights / centers replicated. No cross-device communication.

Per-core kernel design (fp8 DoubleRow conv stack, v2):
  - x and all conv weights quantized host-side to fp8e4 (TRN E4M3, max 240).
    Weights are pre-scaled by a per-layer power of two (up to ~224 max mag)
    to clear e4m3 subnormals; the inverse scale rides the PSUM eviction.
  - conv layer = K/2 tap-pair matmuls in MatmulPerfMode.DoubleRow, which
    virtualizes the PE to a 256-deep contraction (2 fp8 weights/cell):
        out[o, l] += sum_i W[o,:,k+i]^T . h[:, 2l+k+i],  i in {0,1}
    lhsT = (128, 2, 128) tap-pair slice of the packed weights; rhs is the
    contiguous slice h[k : k+2*Lout] rearranged "(l two) -> two l" (the
    stride-2 conv makes tap pairs adjacent in memory). Odd K padded with a
    zero tap (conv1 15->16, conv3 7->8; h2 rows are stride-248-padded so
    the zero tap's read stays in-bounds).
  - conv1-3 eviction is TWO ops (was 3): ACT A = ps*2^-k + b (PSUM->bf16),
    then LeakyReLU in one scalar_tensor_tensor per half:
        h' = (A * 0.1) max A   (exact lrelu since 0.1 < 1)
    half0 on DVE, half1 on Pool - balances both engines well under the PE
    and removes the old DVE serialization that stalled the PE.
  - conv4 eviction runs ENTIRELY on ACT (DVE only does a small reduce):
    ACT Square(ps*s+b) -> zsq f32; ACT Identity(ps*16s+16b) -> fp8 z8
    (position-major for the DR distance); DVE X-reduce zsq -> part.
  - distance: 4 regions of 8 samples; per region: fp32 cn matmul seeds
    1+||c||^2 (row-0 outer product), 30 fp8-DR position-pair chunks of
    -2 z.c, fp32 part x ones matmul adds ||z||^2 (stop).
  - q = reciprocal(d2') row-normalized on DVE, DMA out per region (fp32).
  - PE pre-warm: dummy matmuls during the DMA lead-in so HAM un-throttles
    (0.65 -> 2.4 GHz) before real conv work; first transfers are split
    (w1 taps 0-7, then single samples) so conv1 starts ~2.8us.
  - CAUTION (probe-verified in a previous session): ACT Prelu would fuse
    the whole lrelu eviction into one op but hard-faults the device when
    >=4 cores run it concurrently. Do not use ACT Prelu / fp16 matmuls.

Measured: see test.py (TimelineSim exec-time metric; baseline was 63209 ns).
"""

import os
import sys

import numpy as np
import ml_dtypes

for _p in ("/opt/trn_rl_repo",):
    if _p not in sys.path and os.path.isdir(_p):
        sys.path.insert(0, _p)

import concourse.bacc as bacc  # noqa: E402
import concourse.mybir as mybir  # noqa: E402
import concourse.tile as tile  # noqa: E402
from concourse import bass_utils  # noqa: E402

F8 = mybir.dt.float8e4
HDT = mybir.dt.bfloat16
F32 = mybir.dt.float32
AF = mybir.ActivationFunctionType
OP = mybir.AluOpType
DR = mybir.MatmulPerfMode.DoubleRow

N_CORES = 8
NS = 32          # samples per core
C = 128          # channels
KCENT = 64       # number of centers
LFIN = 59        # final length
D = C * LFIN     # 7552
NREG = 4         # distance regions (8 samples each)
RS = NS // NREG  # 8 samples per region

# (K_real, K_padded, L_in_row_stride, L_out, L_out_row_stride, G samples/mm)
# conv2's output rows carry 1 pad element (248) so conv3's zero tap 7 reads
# in-bounds; the pad is memset once.
CFG = [
    (15, 16, 1024, 505, 505, 1),
    (12, 12, 505, 247, 248, 2),
    (7, 8, 248, 121, 121, 4),
    (4, 4, 121, 59, 59, 8),
]

N_WARM = 30  # PE pre-warm dummy matmuls

_BUILt = {}


def _schedule():
    """Block emission order: 16 conv1 blocks (2 samples), 8 conv2 (4),
    4 conv3 (8), 4 conv4 (8), 4 dist+q regions (8). Deep blocks of group g
    weave between other groups' work so eviction latency hides under PE
    matmuls; the final chain c2:7 -> c3:3 -> c4:3 -> d3 -> q3 is spread
    with the remaining independent blocks."""
    return [
        ("c", 0, 0), ("c", 0, 1), ("c", 0, 2), ("c", 0, 3),
        ("c", 0, 4), ("c", 0, 5),
        ("c", 1, 0),
        ("c", 0, 6), ("c", 0, 7),
        ("c", 1, 1),
        ("c", 0, 8),
        ("c", 2, 0),
        ("c", 0, 9),
        ("c", 1, 2),
        ("c", 0, 10),
        ("c", 3, 0),
        ("c", 1, 3),
        ("c", 0, 11),
        ("d", 0),
        ("c", 2, 1),
        ("q", 0),
        ("c", 0, 12),
        ("c", 1, 4),
        ("c", 3, 1),
        ("c", 0, 13),
        ("d", 1),
        ("c", 1, 5),
        ("c", 2, 2),
        ("q", 1),
        ("c", 0, 14),
        ("c", 1, 6),
        ("c", 3, 2),
        ("c", 0, 15),
        ("d", 2),
        ("c", 1, 7),
        ("c", 2, 3),
        ("q", 2),
        ("c", 3, 3),
        ("d", 3),
        ("q", 3),
    ]


def _check_schedule(sched):
    """Topological sanity: every block's producers appear earlier."""
    seen = set()
    for blk in sched:
        if blk[0] == "c":
            li, p = blk[1], blk[2]
            if li > 0:
                g = 2 if li == 1 else (4 if li == 2 else 8)
                src = 2 if li == 1 else 2  # two source blocks per block
                base = p * 2
                if li == 1:
                    deps = [("c", 0, base), ("c", 0, base + 1)]
                elif li == 2:
                    deps = [("c", 1, base), ("c", 1, base + 1)]
                else:
                    deps = [("c", 2, p)]
                for d in deps:
                    assert d in seen, f"{blk} before {d}"
        elif blk[0] == "d":
            assert ("c", 3, blk[1]) in seen, f"{blk} before conv4"
        else:
            assert ("d", blk[1]) in seen, f"{blk} before dist"
        seen.add(blk)
    for li, n in ((0, 16), (1, 8), (2, 4), (3, 4)):
        for p in range(n):
            assert ("c", li, p) in seen
    for p in range(NREG):
        assert ("d", p) in seen and ("q", p) in seen


def _build_program(n_repeat=1):
    """Build + compile the per-core Bass program (same program on all cores)."""
    nc = bacc.Bacc("TRN2", target_bir_lowering=False, debug=False)

    # ---- DRAM I/O ----
    x_d = nc.dram_tensor("x", (C, NS, 1024), F8, kind="ExternalInput")
    w_d = [
        nc.dram_tensor(f"w{i+1}", (C, CFG[i][1] * C), F8, kind="ExternalInput")
        for i in range(4)
    ]
    # bias/scale pack: cols 0-3 = b1..b4; cols 4-7 = 2^-k1..2^-k4;
    # cols 8-71 = ones (zn mm rhs); cols 72-135 = row0-only 1+||c||^2
    # (cn mm rhs); cols 136-167 = e0 (partition-0 ones, cn mm lhsT);
    # col 168 = 16*2^-k4 (z8 scale); col 169 = 16*b4 (z8 bias)
    bp_d = nc.dram_tensor("bp", (C, 170), F32, kind="ExternalInput")
    # fp8 centers: cr8[c, l*64+j] = (-2/16)*centers[j, c*59+l], chunk 59 = 0
    cr_d = nc.dram_tensor("cr", (C, 60 * KCENT), F8, kind="ExternalInput")
    q_d = nc.dram_tensor("q", (NS, KCENT), F32, kind="ExternalOutput")

    with tile.TileContext(nc) as tc:
        with (
            tc.tile_pool(name="consts", bufs=1) as cpool,
            tc.tile_pool(name="xp", bufs=8) as xpool,
            tc.tile_pool(name="hp", bufs=1) as hpool,
            tc.tile_pool(name="ap", bufs=4) as apool,
            tc.tile_pool(name="small", bufs=1) as mpool,
            tc.tile_pool(name="psA", bufs=3, space="PSUM") as psA,
            tc.tile_pool(name="psD", bufs=2, space="PSUM") as psD,
        ):
            wt = [
                cpool.tile([C, CFG[i][1] * C], F8, tag=f"w{i}", name=f"wt{i}")
                for i in range(4)
            ]
            bp = cpool.tile([C, 170], F32, tag="bp")
            cr = cpool.tile([C, 60 * KCENT], F8, tag="cr")

            for _rep in range(n_repeat):
                _body_once(nc, tc, x_d, q_d, w_d, bp_d, cr_d, wt, bp,
                           cr, xpool, hpool, apool, mpool,
                           psA, psD, load_consts=(_rep == 0))

    nc.compile()
    return nc


def _body_once(nc, tc, x_d, q_d, w_d, bp_d, cr_d, wt, bp, cr,
               xpool, hpool, apool, mpool, psA, psD, load_consts=True):
            # ---- Warm-up scratch FIRST so the PE can start ramping while
            # the DMA lead-in runs ----
            if load_consts:
                wsrc = mpool.tile([1, 128], HDT, tag="warm", name="warm")
                nc.gpsimd.memset(wsrc[:], 0.0)

            # ---- Two HWDGE rings. SP ring: w1 taps 0-7, then x samples 0,1,
            # then 2-sample x chunks (smallest-first so conv1 starts early).
            # ACT ring: w1 taps 8-15, w2-4, bp, cr ----
            w1v = w_d[0].ap().rearrange("p (k o) -> p k o", o=C)
            wt1v = wt[0][:].rearrange("p (k o) -> p k o", o=C)
            if load_consts:
                nc.sync.dma_start(wt1v[:, 0:8, :], w1v[:, 0:8, :])
            x3 = x_d.ap()  # (C, NS, 1024)
            xch = []
            xt = xpool.tile([C, 2 * 1024], F8, tag="x", name="xch0")
            xt3 = xt[:].rearrange("p (a b) -> p a b", a=2)
            nc.sync.dma_start(xt3[:, 0:1, :], x3[:, 0:1, :])
            if load_consts:
                nc.scalar.dma_start(wt1v[:, 8:16, :], w1v[:, 8:16, :])
                nc.scalar.dma_start(bp[:], bp_d.ap())
            nc.sync.dma_start(xt3[:, 1:2, :], x3[:, 1:2, :])
            xch.append(xt)
            for g in range(1, 16):
                t = xpool.tile([C, 2 * 1024], F8, tag="x", name=f"xch{g}")
                src = x3[:, 2 * g : 2 * g + 2, :].rearrange("p a b -> p (a b)")
                nc.sync.dma_start(t[:], src)
                xch.append(t)
            if load_consts:
                for i in range(1, 4):
                    nc.scalar.dma_start(wt[i][:], w_d[i].ap())
                nc.scalar.dma_start(cr[:], cr_d.ap())

                # ---- PE pre-warm: ramp the p-state (0.65 -> 2.4 GHz needs
                # ~3us of continuous busy) while the first DMAs land ----
                wps = psA.tile([C, 128], F32, tag="ps", name="warmps")
                for _w in range(N_WARM):
                    nc.tensor.matmul(
                        wps[:], wsrc[:], wsrc[:],
                        start=(_w == 0), stop=(_w == N_WARM - 1)
                    )

            # ---- shared tiles for the conv stack / distance tail ----
            h_tiles = []
            for li, (K, Kp, Lin, Lout, Lrow, G) in enumerate(CFG[:3]):
                hdst = hpool.tile([C, NS * Lrow], F8, tag=f"h{li}")
                if Lrow > Lout:
                    # zero the per-sample pad so the zero tap's in-bounds
                    # read never multiplies NaN garbage
                    padv = hdst[:].rearrange("p (n l) -> p n l", n=NS)
                    nc.gpsimd.memset(padv[:, :, Lout:Lrow], 0.0)
                h_tiles.append(hdst)
            zsq = hpool.tile([C, NS * LFIN], F32, tag="zsq", name="zsq")
            part = mpool.tile([C, NS], F32, tag="part", name="part")
            z8 = hpool.tile([C, 60 * NS], F8, tag="z8", name="z8")
            nc.gpsimd.memset(z8[:, LFIN * NS : 60 * NS], 0.0)
            z84 = z8[:].rearrange("p (l n) -> p l n", n=NS)
            z83 = z8[:].rearrange("p (l n) -> p l n", n=NS)
            cr3 = cr[:].rearrange("p (l j) -> p l j", j=KCENT)
            dps = [None] * NREG

            def conv_block(li, pr):
                """One PSUM block. conv1-3: two G-sample halves sharing a
                2-bank tile; evict = ACT affine + one lrelu STT per half
                (DVE half0, Pool half1). conv4: one 8-sample group; evict =
                ACT Square -> zsq, ACT Identity -> fp8 z8, DVE reduce."""
                K, Kp, Lin, Lout, Lrow, G = CFG[li]
                if li < 3:
                    hdst3 = h_tiles[li][:].rearrange("p (n l) -> p n l", n=NS)
                if li > 0:
                    hsrc3 = h_tiles[li - 1][:].rearrange("p (n l) -> p n l", n=NS)
                nhalf = 1 if li == 3 else 2
                gp = pr * nhalf * G
                ps = psA.tile([C, 1024], F32, tag="ps")
                for half in range(nhalf):
                    g0 = gp + half * G
                    pslice = ps[:, half * 512 : half * 512 + G * Lout]
                    for kp in range(0, Kp, 2):
                        lhsT = wt[li][:, kp * C : (kp + 2) * C].rearrange(
                            "p (two o) -> p two o", two=2
                        )
                        if li == 0:
                            xv = xch[g0 // 2][:].rearrange("p (a b) -> p a b", a=2)
                            rhs = xv[
                                :, g0 % 2 : g0 % 2 + 1, kp : kp + 2 * Lout
                            ].rearrange("p n (l two) -> p two n l", two=2)
                        else:
                            rhs = hsrc3[
                                :, g0 : g0 + G, kp : kp + 2 * Lout
                            ].rearrange("p n (l two) -> p two n l", two=2)
                        nc.tensor.matmul(
                            pslice, lhsT, rhs,
                            start=(kp == 0), stop=(kp == Kp - 2),
                            perf_mode=DR,
                        )
                bias = bp[:, li : li + 1]
                scale = bp[:, 4 + li : 5 + li]
                ng = nhalf * G
                E = ng * Lout
                if li < 3:
                    # ACT: A = ps*2^-k + b for the whole block, then
                    # lrelu h = (0.1*A) max A, one STT per half
                    psv = ps[:].rearrange("p (g l) -> p g l", g=2)[:, :, 0 : G * Lout]
                    A = apool.tile([C, E], HDT, tag="A")
                    nc.scalar.activation(
                        A[:], psv, AF.Identity, bias=bias, scale=scale
                    )
                    A3 = A[:].rearrange("p (n l) -> p n l", n=ng)
                    hm = ng // 2
                    dsl0 = hdst3[:, gp : gp + hm, 0:Lout]
                    dsl1 = hdst3[:, gp + hm : gp + ng, 0:Lout]
                    nc.vector.scalar_tensor_tensor(
                        dsl0, A3[:, 0:hm, :], 0.1, A3[:, 0:hm, :],
                        op0=OP.mult, op1=OP.max,
                    )
                    nc.gpsimd.scalar_tensor_tensor(
                        dsl1, A3[:, hm:ng, :], 0.1, A3[:, hm:ng, :],
                        op0=OP.mult, op1=OP.max,
                    )
                else:
                    # conv4: all-ACT eviction + small DVE reduce
                    psv = ps[:, 0 : G * Lout]
                    zsl = zsq[:, gp * LFIN : (gp + ng) * LFIN]
                    nc.scalar.activation(
                        zsl, psv, AF.Square, bias=bias, scale=scale
                    )
                    outv = z83[:, 0:LFIN, gp : gp + ng].rearrange("p l n -> p n l")
                    inv = psv.rearrange("p (n l) -> p n l", n=ng)
                    nc.scalar.activation(
                        outv, inv, AF.Identity,
                        bias=bp[:, 169:170], scale=bp[:, 168:169],
                    )
                    nc.vector.tensor_reduce(
                        part[:, gp : gp + ng],
                        zsl.rearrange("p (n l) -> p n l", n=ng),
                        axis=mybir.AxisListType.X,
                        op=OP.add,
                    )

            def dist_block(p):
                """d2 for RS samples in one PSUM tile (partition base 0):
                cn (start) -> 30 fp8-DR position-pair chunks -> zn (stop)."""
                dp = psD.tile([RS, KCENT], F32, tag="d")
                dps[p] = dp
                nc.tensor.matmul(
                    dp[:], bp[:, 136 : 136 + RS], bp[:, 72:136],
                    start=True, stop=False,
                )
                for lp in range(0, 60, 2):
                    lhsT = z84[:, lp : lp + 2, RS * p : RS * p + RS]
                    nc.tensor.matmul(
                        dp[:], lhsT, cr3[:, lp : lp + 2, :],
                        start=False, stop=False, perf_mode=DR,
                    )
                nc.tensor.matmul(
                    dp[:], part[:, RS * p : RS * p + RS], bp[:, 8:72],
                    start=False, stop=True,
                )

            def q_block(p):
                """q = normalize(1/d2') for RS samples; DMA out per region."""
                dp = dps[p]
                qn = mpool.tile([RS, KCENT], F32, tag=f"qn{p}")
                nc.vector.reciprocal(qn[:], dp[:])
                rs = mpool.tile([RS, 1], F32, tag=f"rs{p}")
                nc.vector.tensor_reduce(
                    rs[:], qn[:], axis=mybir.AxisListType.X, op=OP.add
                )
                rr = mpool.tile([RS, 1], F32, tag=f"rr{p}")
                nc.vector.reciprocal(rr[:], rs[:])
                nc.vector.tensor_scalar_mul(qn[:], qn[:], rr[:])
                nc.sync.dma_start(q_d.ap()[RS * p : RS * p + RS, :], qn[:])

            sched = _schedule()
            _check_schedule(sched)
            for blk in sched:
                if blk[0] == "c":
                    conv_block(blk[1], blk[2])
                elif blk[0] == "d":
                    dist_block(blk[1])
                else:
                    q_block(blk[1])


def _get_program(n_repeat=1):
    if n_repeat not in _BUILt:
        _BUILt[n_repeat] = _build_program(n_repeat)
    return _BUILt[n_repeat]


def _to_f8(a):
    """fp32 -> TRN E4M3 (max 240; clip so OCP e4m3fn bit patterns match)."""
    return np.clip(a, -240.0, 240.0).astype(ml_dtypes.float8_e4m3fn)


def _prep_inputs(x, w1, b1, w2, b2, w3, b3, w4, b4, centers):
    """Host-side prep: fp8 quantization, weight transposes, sharding."""
    ws = [w1, w2, w3, w4]
    bs = [b1, b2, b3, b4]

    const_map = {}
    scales = []
    for i, w in enumerate(ws):
        K, Kp = CFG[i][0], CFG[i][1]
        wf = np.asarray(w, np.float32)  # (O, I, K)
        # per-layer power-of-2 scale-up to ~224 max magnitude (e4m3 headroom)
        mx = float(np.abs(wf).max())
        k = int(np.floor(np.log2(224.0 / mx))) if mx > 0 else 0
        scales.append(2.0 ** (-k))
        wq = wf * (2.0 ** k)
        # (O, I, K) -> (I, Kp, O): lhsT tap k = [:, k*128:(k+1)*128]
        wp = np.zeros((C, Kp, C), np.float32)
        wp[:, :K, :] = wq.transpose(1, 2, 0)
        const_map[f"w{i+1}"] = _to_f8(wp.reshape(C, Kp * C))

    cent = np.asarray(centers, np.float32)
    # cr8[c, l*64 + j] = (-2/16) * centers[j, c*59 + l]; position chunk 59
    # is zero (pairs the z8 pad so the DR distance contracts 60 positions).
    # The 1/16 undoes z8's x16 pre-scale (both powers of 2, exact).
    cr8 = np.zeros((C, 60, KCENT), np.float32)
    cr8[:, :LFIN, :] = (
        (-2.0 / 16.0 * cent).reshape(KCENT, C, LFIN).transpose(1, 2, 0)
    )
    const_map["cr"] = _to_f8(cr8.reshape(C, 60 * KCENT))
    cn = 1.0 + (cent.astype(np.float64) ** 2).sum(axis=1)  # (64,)

    bpk = np.zeros((C, 170), np.float32)
    for i, b in enumerate(bs):
        bpk[:, i] = np.asarray(b, np.float32)
        bpk[:, 4 + i] = scales[i]
    bpk[:, 8:72] = 1.0                      # zn mm rhs (ones)
    bpk[0, 72:136] = cn.astype(np.float32)  # cn mm rhs (row 0 only)
    bpk[0, 136:168] = 1.0                   # cn mm lhsT e0 (row 0 only)
    bpk[:, 168] = 16.0 * scales[3]          # z8 scale
    bpk[:, 169] = 16.0 * np.asarray(bs[3], np.float32)  # z8 bias
    const_map["bp"] = bpk

    xf = np.asarray(x, np.float32)
    in_maps = []
    for c in range(N_CORES):
        shard = xf[c * NS : (c + 1) * NS]  # (32, 128, 1024)
        xc = _to_f8(np.ascontiguousarray(shard.transpose(1, 0, 2)))  # (128,32,1024)
        in_maps.append({"x": xc, **const_map})
    return in_maps


def _ensure_devices():
    """Absorb wedged-device attach faults with a tiny op before the real run.

    A previous process can leave a NeuronCore wedged
    (NRT_EXEC_UNIT_UNRECOVERABLE); the first attach after a wedge fails and
    triggers a reset that completes within ~60 s.
    """
    import time

    import jax
    import jax.numpy as jnp

    for attempt in range(3):
        try:
            outs = [jax.device_put(jnp.zeros((8,)), d) + 1.0 for d in jax.devices()]
            jax.block_until_ready(outs)
            return
        except Exception:  # noqa: BLE001 - device fault; wait out the reset
            if attempt == 2:
                raise
            time.sleep(60)


def run(trace=False, **inputs):
    """Run the kernel; returns (q_full, BassKernelResults).

    Retries on device-unrecoverable faults (see _ensure_devices).
    """
    import time

    _ensure_devices()
    nc = _get_program()
    in_maps = _prep_inputs(**inputs)
    last_err = None
    for attempt in range(3):
        try:
            res = bass_utils.run_bass_kernel_spmd(
                nc, in_maps, core_ids=list(range(N_CORES)), trace=trace
            )
            break
        except Exception as e:  # noqa: BLE001 - device fault, wait + retry
            last_err = e
            if "UNAVAILABLE" not in str(e) and "unrecoverable" not in str(e).lower():
                raise
            time.sleep(60)
    else:
        raise last_err
    q = np.concatenate([res.results[c]["q"] for c in range(N_CORES)], axis=0)
    return np.ascontiguousarray(q.astype(np.float32)), res


def kernel(**inputs) -> np.ndarray:
    q, _ = run(trace=False, **inputs)
    return q


# revision 15
# speedup vs baseline: 1.0776x; 1.0568x over previous
"""Trainium2 Bass kernel for nn_DEC_26139170963600 (vq_codebook).

Reference computation:
  4x strided conv1d (stride 2, VALID) with LeakyReLU(0.1) between layers,
  flatten -> soft VQ assignment over 64 centers:
      d2 = ||z||^2 + ||c||^2 - 2 z.c
      q  = (1/(1+d2)) row-normalized            (alpha=1 -> exponent is 1)

Sharding: data-parallel over batch N=256 across 8 cores (32 samples/core).
Weights / centers replicated. No cross-device communication.

Per-core kernel design (fp8 DoubleRow conv stack, v2):
  - x and all conv weights quantized host-side to fp8e4 (TRN E4M3, max 240).
    Weights are pre-scaled by a per-layer power of two (up to ~224 max mag)
    to clear e4m3 subnormals; the inverse scale rides the PSUM eviction.
  - conv layer = K/2 tap-pair matmuls in MatmulPerfMode.DoubleRow, which
    virtualizes the PE to a 256-deep contraction (2 fp8 weights/cell):
        out[o, l] += sum_i W[o,:,k+i]^T . h[:, 2l+k+i],  i in {0,1}
    lhsT = (128, 2, 128) tap-pair slice of the packed weights; rhs is the
    contiguous slice h[k : k+2*Lout] rearranged "(l two) -> two l" (the
    stride-2 conv makes tap pairs adjacent in memory). Odd K padded with a
    zero tap (conv1 15->16, conv3 7->8; h2 rows are stride-248-padded so
    the zero tap's read stays in-bounds).
  - conv1-3 eviction is TWO ops (was 3): ACT A = ps*2^-k + b (PSUM->bf16),
    then LeakyReLU in one scalar_tensor_tensor per half:
        h' = (A * 0.1) max A   (exact lrelu since 0.1 < 1)
    half0 on DVE, half1 on Pool - balances both engines well under the PE
    and removes the old DVE serialization that stalled the PE.
  - conv4 eviction runs ENTIRELY on ACT (DVE only does a small reduce):
    ACT Square(ps*s+b) -> zsq f32; ACT Identity(ps*16s+16b) -> fp8 z8
    (position-major for the DR distance); DVE X-reduce zsq -> part.
  - distance: 4 regions of 8 samples; per region: fp32 cn matmul seeds
    1+||c||^2 (row-0 outer product), 30 fp8-DR position-pair chunks of
    -2 z.c, fp32 part x ones matmul adds ||z||^2 (stop).
  - q = reciprocal(d2') row-normalized on DVE, DMA out per region (fp32).
  - PE pre-warm: dummy matmuls during the DMA lead-in so HAM un-throttles
    (0.65 -> 2.4 GHz) before real conv work; first transfers are split
    (w1 taps 0-7, then single samples) so conv1 starts ~2.8us.
  - CAUTION (probe-verified in a previous session): ACT Prelu would fuse
    the whole lrelu eviction into one op but hard-faults the device when
    >=4 cores run it concurrently. Do not use ACT Prelu / fp16 matmuls.

Measured: see test.py (TimelineSim exec-time metric; baseline was 63209 ns).
"""

import os
import sys

import numpy as np
import ml_dtypes

for _p in ("/opt/trn_rl_repo",):
    if _p not in sys.path and os.path.isdir(_p):
        sys.path.insert(0, _p)

import concourse.bacc as bacc  # noqa: E402
import concourse.mybir as mybir  # noqa: E402
import concourse.tile as tile  # noqa: E402
from concourse import bass_utils  # noqa: E402

F8 = mybir.dt.float8e4
HDT = mybir.dt.bfloat16
F32 = mybir.dt.float32
AF = mybir.ActivationFunctionType
OP = mybir.AluOpType
DR = mybir.MatmulPerfMode.DoubleRow

N_CORES = 8
NS = 32          # samples per core
C = 128          # channels
KCENT = 64       # number of centers
LFIN = 59        # final length
D = C * LFIN     # 7552
NREG = 4         # distance regions (8 samples each)
RS = NS // NREG  # 8 samples per region

# (K_real, K_padded, L_in_row_stride, L_out, L_out_row_stride, G samples/mm)
# conv2's output rows carry 1 pad element (248) so conv3's zero tap 7 reads
# in-bounds; the pad is memset once.
CFG = [
    (15, 16, 1024, 505, 505, 1),
    (12, 12, 505, 247, 248, 2),
    (7, 8, 248, 121, 121, 4),
    (4, 4, 121, 59, 59, 8),
]

N_WARM = 25  # PE pre-warm dummy matmuls
# lrelu STT column split per half: DVE takes the first SPLIT_FRAC of each
# half's columns, Pool the rest (both run concurrently)
SPLIT_L = {0: 278, 1: 136, 2: 67}

_BUILt = {}


def _schedule():
    """Block emission order. Entries:
      ("c", li, g0, ng, fast)  conv block over samples [g0, g0+ng)
      ("d", didx, g0, rs)      distance region
      ("q", didx, g0, rs)      q normalization + DMA out
    Bulk blocks (fast=False) use the throughput eviction; the final chain
    over samples 24-31 is tapered into 2-sample pieces with low-latency
    (fast=True) evictions so the pipeline drain is short."""
    c1 = [("c", 0, 2 * p, 2, False) for p in range(16)]
    c2 = [("c", 1, 4 * p, 4, False) for p in range(7)]
    c3 = [("c", 2, 8 * p, 8, False) for p in range(3)]
    c4 = [("c", 3, 8 * p, 8, False) for p in range(3)]
    return [
        c1[0], c1[1], c1[2], c1[3],
        c1[4], c1[5],
        c2[0],
        c1[6], c1[7],
        c2[1],
        c1[8],
        c3[0],
        c1[9],
        c2[2],
        c1[10],
        c4[0],
        c2[3],
        c1[11],
        ("d", 0, 0, 8),
        c3[1],
        ("q", 0, 0, 8),
        c1[12],
        c2[4],
        c4[1],
        c1[13],
        ("d", 1, 8, 8),
        c2[5],
        c1[14],
        c3[2],
        ("q", 1, 8, 8),
        c1[15],
        c2[6],
        c4[2],
        ("c", 1, 28, 2, True),
        ("d", 2, 16, 8),
        ("c", 1, 30, 2, True),
        ("c", 2, 24, 4, True),
        ("q", 2, 16, 8),
        ("c", 2, 28, 2, True),
        ("c", 2, 30, 2, True),
        ("c", 3, 24, 4, False),
        ("d", 3, 24, 4),
        ("c", 3, 28, 2, True),
        ("q", 3, 24, 4),
        ("d", 4, 28, 2),
        ("c", 3, 30, 2, True),
        ("q", 4, 28, 2),
        ("d", 5, 30, 2),
        ("q", 5, 30, 2),
    ]


def _check_schedule(sched):
    """Topological sanity: every block's producing sample ranges appear
    earlier, and all samples are covered exactly once per stage."""
    done = {li: set() for li in range(4)}
    ddone = {}
    for blk in sched:
        if blk[0] == "c":
            _, li, g0, ng, _fast = blk
            rng = set(range(g0, g0 + ng))
            assert not (rng & done[li]), f"dup {blk}"
            if li > 0:
                assert rng <= done[li - 1], f"{blk} missing producer samples"
            done[li] |= rng
        elif blk[0] == "d":
            _, didx, g0, rs = blk
            rng = set(range(g0, g0 + rs))
            assert rng <= done[3], f"{blk} before conv4"
            ddone[didx] = rng
        else:
            _, didx, g0, rs = blk
            assert ddone.get(didx) == set(range(g0, g0 + rs)), f"{blk} vs dist"
    for li in range(4):
        assert done[li] == set(range(NS)), f"layer {li} incomplete"
    assert set().union(*ddone.values()) == set(range(NS))


def _build_program(n_repeat=1):
    """Build + compile the per-core Bass program (same program on all cores)."""
    nc = bacc.Bacc("TRN2", target_bir_lowering=False, debug=False)

    # ---- DRAM I/O ----
    x_d = nc.dram_tensor("x", (C, NS, 1024), F8, kind="ExternalInput")
    w_d = [
        nc.dram_tensor(f"w{i+1}", (C, CFG[i][1] * C), F8, kind="ExternalInput")
        for i in range(4)
    ]
    # bias/scale pack: cols 0-3 = b1..b4; cols 4-7 = 2^-k1..2^-k4;
    # cols 8-71 = ones (zn mm rhs); cols 72-135 = row0-only 1+||c||^2
    # (cn mm rhs); cols 136-167 = e0 (partition-0 ones, cn mm lhsT);
    # col 168 = 16*2^-k4 (z8 scale); col 169 = 16*b4 (z8 bias)
    bp_d = nc.dram_tensor("bp", (C, 170), F32, kind="ExternalInput")
    # fp8 centers: cr8[c, l*64+j] = (-2/16)*centers[j, c*59+l], chunk 59 = 0
    cr_d = nc.dram_tensor("cr", (C, 60 * KCENT), F8, kind="ExternalInput")
    q_d = nc.dram_tensor("q", (NS, KCENT), F32, kind="ExternalOutput")

    with tile.TileContext(nc) as tc:
        with (
            tc.tile_pool(name="consts", bufs=1) as cpool,
            tc.tile_pool(name="xp", bufs=8) as xpool,
            tc.tile_pool(name="hp", bufs=1) as hpool,
            tc.tile_pool(name="ap", bufs=4) as apool,
            tc.tile_pool(name="small", bufs=1) as mpool,
            tc.tile_pool(name="psA", bufs=3, space="PSUM") as psA,
            tc.tile_pool(name="psD", bufs=2, space="PSUM") as psD,
        ):
            wt = [
                cpool.tile([C, CFG[i][1] * C], F8, tag=f"w{i}", name=f"wt{i}")
                for i in range(4)
            ]
            bp = cpool.tile([C, 170], F32, tag="bp")
            cr = cpool.tile([C, 60 * KCENT], F8, tag="cr")

            for _rep in range(n_repeat):
                _body_once(nc, tc, x_d, q_d, w_d, bp_d, cr_d, wt, bp,
                           cr, xpool, hpool, apool, mpool,
                           psA, psD, load_consts=(_rep == 0))

    nc.compile()
    return nc


def _body_once(nc, tc, x_d, q_d, w_d, bp_d, cr_d, wt, bp, cr,
               xpool, hpool, apool, mpool, psA, psD, load_consts=True):
            # ---- Warm-up scratch FIRST so the PE can start ramping while
            # the DMA lead-in runs ----
            if load_consts:
                wsrc = mpool.tile([1, 128], HDT, tag="warm", name="warm")
                nc.gpsimd.memset(wsrc[:], 0.0)

            # ---- Two HWDGE rings. SP ring: w1 taps 0-7, then x samples 0,1,
            # then 2-sample x chunks (smallest-first so conv1 starts early).
            # ACT ring: w1 taps 8-15, bp early; w2-4, cr after the first few
            # x chunks so they don't delay conv1's input stream ----
            w1v = w_d[0].ap().rearrange("p (k o) -> p k o", o=C)
            wt1v = wt[0][:].rearrange("p (k o) -> p k o", o=C)
            if load_consts:
                nc.sync.dma_start(wt1v[:, 0:8, :], w1v[:, 0:8, :])
            x3 = x_d.ap()  # (C, NS, 1024)
            xch = []
            xt = xpool.tile([C, 2 * 1024], F8, tag="x", name="xch0")
            xt3 = xt[:].rearrange("p (a b) -> p a b", a=2)
            nc.sync.dma_start(xt3[:, 0:1, :], x3[:, 0:1, :])
            if load_consts:
                nc.scalar.dma_start(wt1v[:, 8:16, :], w1v[:, 8:16, :])
                nc.scalar.dma_start(bp[:], bp_d.ap())
            nc.sync.dma_start(xt3[:, 1:2, :], x3[:, 1:2, :])
            xch.append(xt)
            for g in range(1, 16):
                t = xpool.tile([C, 2 * 1024], F8, tag="x", name=f"xch{g}")
                src = x3[:, 2 * g : 2 * g + 2, :].rearrange("p a b -> p (a b)")
                nc.sync.dma_start(t[:], src)
                xch.append(t)
                if load_consts and g == 3:
                    for i in range(1, 4):
                        nc.scalar.dma_start(wt[i][:], w_d[i].ap())
                    nc.scalar.dma_start(cr[:], cr_d.ap())

            # ---- PE pre-warm: ramp the p-state (0.65 -> 2.4 GHz needs
            # ~3us of continuous busy) while the first DMAs land ----
            if load_consts:
                wps = psD.tile([C, 128], F32, tag="d", name="warmps")
                for _w in range(N_WARM):
                    nc.tensor.matmul(
                        wps[:], wsrc[:], wsrc[:],
                        start=(_w == 0), stop=(_w == N_WARM - 1)
                    )

            # ---- shared tiles for the conv stack / distance tail ----
            h_tiles = []
            for li, (K, Kp, Lin, Lout, Lrow, G) in enumerate(CFG[:3]):
                hdst = hpool.tile([C, NS * Lrow], F8, tag=f"h{li}")
                if Lrow > Lout:
                    # zero the per-sample pad so the zero tap's in-bounds
                    # read never multiplies NaN garbage
                    padv = hdst[:].rearrange("p (n l) -> p n l", n=NS)
                    nc.gpsimd.memset(padv[:, :, Lout:Lrow], 0.0)
                h_tiles.append(hdst)
            zsq = hpool.tile([C, NS * LFIN], HDT, tag="zsq", name="zsq")
            part = mpool.tile([C, NS], HDT, tag="part", name="part")
            ones = mpool.tile([C, KCENT], HDT, tag="ones", name="ones")
            nc.gpsimd.memset(ones[:], 1.0)
            z8 = hpool.tile([C, 60 * NS], F8, tag="z8", name="z8")
            nc.gpsimd.memset(z8[:, LFIN * NS : 60 * NS], 0.0)
            z84 = z8[:].rearrange("p (l n) -> p l n", n=NS)
            cr3 = cr[:].rearrange("p (l j) -> p l j", j=KCENT)
            dps = [None] * 8

            def conv_block(li, gp, ng, fast):
                """One PSUM block over samples [gp, gp+ng). Pieces of up to
                G samples (<=512 PSUM cols) accumulate into one tile.
                conv1-3 eviction:
                  std:  one ACT affine over the block, then one lrelu STT
                        per piece (DVE piece0, Pool piece1)
                  fast: per-piece ACT + column-split STT on DVE||Pool
                        (lowest latency; used on the tapered tail chain)
                conv4: ACT Identity -> fp8 z8, ACT Square -> bf16 zsq,
                DVE X-reduce -> part (all per block)."""
                K, Kp, Lin, Lout, Lrow, G = CFG[li]
                G = min(G, ng)
                npc = (ng + G - 1) // G
                if li < 3:
                    hdst3 = h_tiles[li][:].rearrange("p (n l) -> p n l", n=NS)
                if li > 0:
                    hsrc3 = h_tiles[li - 1][:].rearrange("p (n l) -> p n l", n=NS)
                ps = psA.tile([C, 1024], F32, tag="ps")
                for pc in range(npc):
                    g0 = gp + pc * G
                    pslice = ps[:, pc * 512 : pc * 512 + G * Lout]
                    for kp in range(0, Kp, 2):
                        lhsT = wt[li][:, kp * C : (kp + 2) * C].rearrange(
                            "p (two o) -> p two o", two=2
                        )
                        if li == 0:
                            xv = xch[g0 // 2][:].rearrange("p (a b) -> p a b", a=2)
                            rhs = xv[
                                :, g0 % 2 : g0 % 2 + 1, kp : kp + 2 * Lout
                            ].rearrange("p n (l two) -> p two n l", two=2)
                        else:
                            rhs = hsrc3[
                                :, g0 : g0 + G, kp : kp + 2 * Lout
                            ].rearrange("p n (l two) -> p two n l", two=2)
                        nc.tensor.matmul(
                            pslice, lhsT, rhs,
                            start=(kp == 0), stop=(kp == Kp - 2),
                            perf_mode=DR,
                        )
                bias = bp[:, li : li + 1]
                scale = bp[:, 4 + li : 5 + li]
                if li < 3:
                    if not fast:
                        psv = (
                            ps[:].rearrange("p (g l) -> p g l", g=npc)[
                                :, :, 0 : G * Lout
                            ]
                            if npc > 1
                            else ps[:, 0 : G * Lout]
                        )
                        A = apool.tile([C, ng * Lout], HDT, tag="A")
                        nc.scalar.activation(
                            A[:], psv, AF.Identity, bias=bias, scale=scale
                        )
                        A3 = A[:].rearrange("p (n l) -> p n l", n=ng)
                        for pc in range(npc):
                            s0, s1 = pc * G, (pc + 1) * G
                            dsl = hdst3[:, gp + s0 : gp + s1, 0:Lout]
                            eng = nc.vector if pc % 2 == 0 else nc.gpsimd
                            eng.scalar_tensor_tensor(
                                dsl, A3[:, s0:s1, :], 0.1, A3[:, s0:s1, :],
                                op0=OP.mult, op1=OP.max,
                            )
                    else:
                        SPL = SPLIT_L[li]
                        for pc in range(npc):
                            g0 = gp + pc * G
                            psh = ps[:, pc * 512 : pc * 512 + G * Lout]
                            A = apool.tile([C, G * Lout], HDT, tag="Af")
                            nc.scalar.activation(
                                A[:], psh, AF.Identity, bias=bias, scale=scale
                            )
                            A3 = A[:].rearrange("p (n l) -> p n l", n=G)
                            dsl = hdst3[:, g0 : g0 + G, 0:Lout]
                            nc.vector.scalar_tensor_tensor(
                                dsl[:, :, 0:SPL], A3[:, :, 0:SPL], 0.1,
                                A3[:, :, 0:SPL], op0=OP.mult, op1=OP.max,
                            )
                            nc.gpsimd.scalar_tensor_tensor(
                                dsl[:, :, SPL:Lout], A3[:, :, SPL:Lout], 0.1,
                                A3[:, :, SPL:Lout], op0=OP.mult, op1=OP.max,
                            )
                else:
                    # conv4 (always one piece: ng*59 <= 472).
                    # std: ACT does both z8 and Square; fast: z8 moves to a
                    # DVE tensor_scalar so ACT's two ops don't serialize on
                    # the tail critical path.
                    psv = ps[:, 0 : ng * Lout]
                    outv = z84[:, 0:LFIN, gp : gp + ng].rearrange("p l n -> p n l")
                    inv = psv.rearrange("p (n l) -> p n l", n=ng)
                    if fast:
                        nc.vector.tensor_scalar(
                            outv, inv, bp[:, 168:169], bp[:, 169:170],
                            op0=OP.mult, op1=OP.add,
                        )
                    else:
                        nc.scalar.activation(
                            outv, inv, AF.Identity,
                            bias=bp[:, 169:170], scale=bp[:, 168:169],
                        )
                    zsl = zsq[:, gp * LFIN : (gp + ng) * LFIN]
                    nc.scalar.activation(
                        zsl, psv, AF.Square, bias=bias, scale=scale
                    )
                    with nc.allow_low_precision(
                        "||z||^2 in bf16: ~2^-8 relative on d2's largest "
                        "term, well inside the 2e-2 gate"
                    ):
                        nc.vector.tensor_reduce(
                            part[:, gp : gp + ng],
                            zsl.rearrange("p (n l) -> p n l", n=ng),
                            axis=mybir.AxisListType.X,
                            op=OP.add,
                        )

            def dist_block(didx, g0, rs):
                """d2 for rs samples in one PSUM tile (partition base 0):
                cn (start) -> 30 fp8-DR position-pair chunks -> zn (stop)."""
                dp = psD.tile([rs, KCENT], F32, tag="d")
                dps[didx] = dp
                nc.tensor.matmul(
                    dp[:], bp[:, 136 : 136 + rs], bp[:, 72:136],
                    start=True, stop=False,
                )
                for lp in range(0, 60, 2):
                    lhsT = z84[:, lp : lp + 2, g0 : g0 + rs]
                    nc.tensor.matmul(
                        dp[:], lhsT, cr3[:, lp : lp + 2, :],
                        start=False, stop=False, perf_mode=DR,
                    )
                nc.tensor.matmul(
                    dp[:], part[:, g0 : g0 + rs], ones[:],
                    start=False, stop=True,
                )

            def q_block(didx, g0, rs):
                """q = normalize(1/d2') for rs samples; DMA out per region."""
                dp = dps[didx]
                qn = mpool.tile([rs, KCENT], F32, tag=f"qn{didx}")
                nc.vector.reciprocal(qn[:], dp[:])
                rsum = mpool.tile([rs, 1], F32, tag=f"rs{didx}")
                nc.vector.tensor_reduce(
                    rsum[:], qn[:], axis=mybir.AxisListType.X, op=OP.add
                )
                rr = mpool.tile([rs, 1], F32, tag=f"rr{didx}")
                nc.vector.reciprocal(rr[:], rsum[:])
                nc.vector.tensor_scalar_mul(qn[:], qn[:], rr[:])
                nc.sync.dma_start(q_d.ap()[g0 : g0 + rs, :], qn[:])

            sched = _schedule()
            _check_schedule(sched)
            for blk in sched:
                if blk[0] == "c":
                    conv_block(blk[1], blk[2], blk[3], blk[4])
                elif blk[0] == "d":
                    dist_block(blk[1], blk[2], blk[3])
                else:
                    q_block(blk[1], blk[2], blk[3])


def _get_program(n_repeat=1):
    if n_repeat not in _BUILt:
        _BUILt[n_repeat] = _build_program(n_repeat)
    return _BUILt[n_repeat]


def _to_f8(a):
    """fp32 -> TRN E4M3 (max 240; clip so OCP e4m3fn bit patterns match)."""
    return np.clip(a, -240.0, 240.0).astype(ml_dtypes.float8_e4m3fn)


def _prep_inputs(x, w1, b1, w2, b2, w3, b3, w4, b4, centers):
    """Host-side prep: fp8 quantization, weight transposes, sharding."""
    ws = [w1, w2, w3, w4]
    bs = [b1, b2, b3, b4]

    const_map = {}
    scales = []
    for i, w in enumerate(ws):
        K, Kp = CFG[i][0], CFG[i][1]
        wf = np.asarray(w, np.float32)  # (O, I, K)
        # per-layer power-of-2 scale-up to ~224 max magnitude (e4m3 headroom)
        mx = float(np.abs(wf).max())
        k = int(np.floor(np.log2(224.0 / mx))) if mx > 0 else 0
        scales.append(2.0 ** (-k))
        wq = wf * (2.0 ** k)
        # (O, I, K) -> (I, Kp, O): lhsT tap k = [:, k*128:(k+1)*128]
        wp = np.zeros((C, Kp, C), np.float32)
        wp[:, :K, :] = wq.transpose(1, 2, 0)
        const_map[f"w{i+1}"] = _to_f8(wp.reshape(C, Kp * C))

    cent = np.asarray(centers, np.float32)
    # cr8[c, l*64 + j] = (-2/16) * centers[j, c*59 + l]; position chunk 59
    # is zero (pairs the z8 pad so the DR distance contracts 60 positions).
    # The 1/16 undoes z8's x16 pre-scale (both powers of 2, exact).
    cr8 = np.zeros((C, 60, KCENT), np.float32)
    cr8[:, :LFIN, :] = (
        (-2.0 / 16.0 * cent).reshape(KCENT, C, LFIN).transpose(1, 2, 0)
    )
    const_map["cr"] = _to_f8(cr8.reshape(C, 60 * KCENT))
    cn = 1.0 + (cent.astype(np.float64) ** 2).sum(axis=1)  # (64,)

    bpk = np.zeros((C, 170), np.float32)
    for i, b in enumerate(bs):
        bpk[:, i] = np.asarray(b, np.float32)
        bpk[:, 4 + i] = scales[i]
    bpk[:, 8:72] = 1.0                      # zn mm rhs (ones)
    bpk[0, 72:136] = cn.astype(np.float32)  # cn mm rhs (row 0 only)
    bpk[0, 136:168] = 1.0                   # cn mm lhsT e0 (row 0 only)
    bpk[:, 168] = 16.0 * scales[3]          # z8 scale
    bpk[:, 169] = 16.0 * np.asarray(bs[3], np.float32)  # z8 bias
    const_map["bp"] = bpk

    xf = np.asarray(x, np.float32)
    in_maps = []
    for c in range(N_CORES):
        shard = xf[c * NS : (c + 1) * NS]  # (32, 128, 1024)
        xc = _to_f8(np.ascontiguousarray(shard.transpose(1, 0, 2)))  # (128,32,1024)
        in_maps.append({"x": xc, **const_map})
    return in_maps


def _ensure_devices():
    """Absorb wedged-device attach faults with a tiny op before the real run.

    A previous process can leave a NeuronCore wedged
    (NRT_EXEC_UNIT_UNRECOVERABLE); the first attach after a wedge fails and
    triggers a reset that completes within ~60 s.
    """
    import time

    import jax
    import jax.numpy as jnp

    for attempt in range(3):
        try:
            outs = [jax.device_put(jnp.zeros((8,)), d) + 1.0 for d in jax.devices()]
            jax.block_until_ready(outs)
            return
        except Exception:  # noqa: BLE001 - device fault; wait out the reset
            if attempt == 2:
                raise
            time.sleep(60)


def run(trace=False, **inputs):
    """Run the kernel; returns (q_full, BassKernelResults).

    Retries on device-unrecoverable faults (see _ensure_devices).
    """
    import time

    _ensure_devices()
    nc = _get_program()
    in_maps = _prep_inputs(**inputs)
    last_err = None
    for attempt in range(3):
        try:
            res = bass_utils.run_bass_kernel_spmd(
                nc, in_maps, core_ids=list(range(N_CORES)), trace=trace
            )
            break
        except Exception as e:  # noqa: BLE001 - device fault, wait + retry
            last_err = e
            if "UNAVAILABLE" not in str(e) and "unrecoverable" not in str(e).lower():
                raise
            time.sleep(60)
    else:
        raise last_err
    q = np.concatenate([res.results[c]["q"] for c in range(N_CORES)], axis=0)
    return np.ascontiguousarray(q.astype(np.float32)), res


def kernel(**inputs) -> np.ndarray:
    q, _ = run(trace=False, **inputs)
    return q


# revision 18
# speedup vs baseline: 1.0851x; 1.0070x over previous
"""Trainium2 Bass kernel for nn_DEC_26139170963600 (vq_codebook).

Reference computation:
  4x strided conv1d (stride 2, VALID) with LeakyReLU(0.1) between layers,
  flatten -> soft VQ assignment over 64 centers:
      d2 = ||z||^2 + ||c||^2 - 2 z.c
      q  = (1/(1+d2)) row-normalized            (alpha=1 -> exponent is 1)

Sharding: data-parallel over batch N=256 across 8 cores (32 samples/core).
Weights / centers replicated. No cross-device communication.

Per-core kernel design (fp8 DoubleRow conv stack, v2):
  - x and all conv weights quantized host-side to fp8e4 (TRN E4M3, max 240).
    Weights are pre-scaled by a per-layer power of two (up to ~224 max mag)
    to clear e4m3 subnormals; the inverse scale rides the PSUM eviction.
  - conv layer = K/2 tap-pair matmuls in MatmulPerfMode.DoubleRow, which
    virtualizes the PE to a 256-deep contraction (2 fp8 weights/cell):
        out[o, l] += sum_i W[o,:,k+i]^T . h[:, 2l+k+i],  i in {0,1}
    lhsT = (128, 2, 128) tap-pair slice of the packed weights; rhs is the
    contiguous slice h[k : k+2*Lout] rearranged "(l two) -> two l" (the
    stride-2 conv makes tap pairs adjacent in memory). Odd K padded with a
    zero tap (conv1 15->16, conv3 7->8; h2 rows are stride-248-padded so
    the zero tap's read stays in-bounds).
  - conv1-3 eviction is TWO ops (was 3): ACT A = ps*2^-k + b (PSUM->bf16),
    then LeakyReLU in one scalar_tensor_tensor per half:
        h' = (A * 0.1) max A   (exact lrelu since 0.1 < 1)
    half0 on DVE, half1 on Pool - balances both engines well under the PE
    and removes the old DVE serialization that stalled the PE.
  - conv4 eviction runs ENTIRELY on ACT (DVE only does a small reduce):
    ACT Square(ps*s+b) -> zsq f32; ACT Identity(ps*16s+16b) -> fp8 z8
    (position-major for the DR distance); DVE X-reduce zsq -> part.
  - distance: 4 regions of 8 samples; per region: fp32 cn matmul seeds
    1+||c||^2 (row-0 outer product), 30 fp8-DR position-pair chunks of
    -2 z.c, fp32 part x ones matmul adds ||z||^2 (stop).
  - q = reciprocal(d2') row-normalized on DVE, DMA out per region (fp32).
  - PE pre-warm: dummy matmuls during the DMA lead-in so HAM un-throttles
    (0.65 -> 2.4 GHz) before real conv work; first transfers are split
    (w1 taps 0-7, then single samples) so conv1 starts ~2.8us.
  - CAUTION (probe-verified in a previous session): ACT Prelu would fuse
    the whole lrelu eviction into one op but hard-faults the device when
    >=4 cores run it concurrently. Do not use ACT Prelu / fp16 matmuls.

Measured: see test.py (TimelineSim exec-time metric; baseline was 63209 ns).
"""

import os
import sys

import numpy as np
import ml_dtypes

for _p in ("/opt/trn_rl_repo",):
    if _p not in sys.path and os.path.isdir(_p):
        sys.path.insert(0, _p)

import concourse.bacc as bacc  # noqa: E402
import concourse.mybir as mybir  # noqa: E402
import concourse.tile as tile  # noqa: E402
from concourse import bass_utils  # noqa: E402

F8 = mybir.dt.float8e4
HDT = mybir.dt.bfloat16
F32 = mybir.dt.float32
AF = mybir.ActivationFunctionType
OP = mybir.AluOpType
DR = mybir.MatmulPerfMode.DoubleRow

N_CORES = 8
NS = 32          # samples per core
C = 128          # channels
KCENT = 64       # number of centers
LFIN = 59        # final length
D = C * LFIN     # 7552
NREG = 4         # distance regions (8 samples each)
RS = NS // NREG  # 8 samples per region

# (K_real, K_padded, L_in_row_stride, L_out, L_out_row_stride, G samples/mm)
# conv2's output rows carry 1 pad element (248) so conv3's zero tap 7 reads
# in-bounds; the pad is memset once.
CFG = [
    (15, 16, 1024, 505, 505, 1),
    (12, 12, 505, 247, 248, 2),
    (7, 8, 248, 121, 121, 4),
    (4, 4, 121, 59, 59, 8),
]

N_WARM = 25  # PE pre-warm dummy matmuls
# lrelu STT column split per half: DVE takes the first SPLIT_FRAC of each
# half's columns, Pool the rest (both run concurrently)
SPLIT_L = {0: 278, 1: 136, 2: 67}

_BUILt = {}


def _schedule():
    """Block emission order. Entries:
      ("c", li, g0, ng, fast)  conv block over samples [g0, g0+ng)
      ("d", didx, g0, rs)      distance region
      ("q", didx, g0, rs)      q normalization + DMA out
    Bulk blocks (fast=False) use the throughput eviction; the final chain
    over samples 24-31 is tapered into 2-sample pieces with low-latency
    (fast=True) evictions so the pipeline drain is short."""
    c1 = [("c", 0, 2 * p, 2, False) for p in range(16)]
    c2 = [("c", 1, 4 * p, 4, False) for p in range(7)]
    c3 = [("c", 2, 8 * p, 8, False) for p in range(3)]
    c4 = [("c", 3, 8 * p, 8, False) for p in range(3)]
    return [
        c1[0], c1[1], c1[2], c1[3],
        c1[4], c1[5],
        c2[0],
        c1[6], c1[7],
        c2[1],
        c1[8],
        c3[0],
        c1[9],
        c2[2],
        c1[10],
        c4[0],
        c2[3],
        c1[11],
        ("d", 0, 0, 8),
        c3[1],
        ("q", 0, 0, 8),
        c1[12],
        c2[4],
        c4[1],
        c1[13],
        ("d", 1, 8, 8),
        c2[5],
        c1[14],
        c3[2],
        ("q", 1, 8, 8),
        c1[15],
        c2[6],
        c4[2],
        ("c", 1, 28, 2, True),
        ("d", 2, 16, 8),
        ("c", 1, 30, 2, True),
        ("c", 2, 24, 4, True),
        ("q", 2, 16, 8),
        ("c", 2, 28, 2, True),
        ("c", 2, 30, 2, True),
        ("c", 3, 24, 4, False),
        ("d", 3, 24, 4),
        ("c", 3, 28, 2, True),
        ("q", 3, 24, 4),
        ("c", 3, 30, 2, True),
        ("d", 4, 28, 4),
        ("q", 4, 28, 4),
    ]


def _check_schedule(sched):
    """Topological sanity: every block's producing sample ranges appear
    earlier, and all samples are covered exactly once per stage."""
    done = {li: set() for li in range(4)}
    ddone = {}
    for blk in sched:
        if blk[0] == "c":
            _, li, g0, ng, _fast = blk
            rng = set(range(g0, g0 + ng))
            assert not (rng & done[li]), f"dup {blk}"
            if li > 0:
                assert rng <= done[li - 1], f"{blk} missing producer samples"
            done[li] |= rng
        elif blk[0] == "d":
            _, didx, g0, rs = blk
            rng = set(range(g0, g0 + rs))
            assert rng <= done[3], f"{blk} before conv4"
            ddone[didx] = rng
        else:
            _, didx, g0, rs = blk
            assert ddone.get(didx) == set(range(g0, g0 + rs)), f"{blk} vs dist"
    for li in range(4):
        assert done[li] == set(range(NS)), f"layer {li} incomplete"
    assert set().union(*ddone.values()) == set(range(NS))


def _build_program(n_repeat=1):
    """Build + compile the per-core Bass program (same program on all cores)."""
    nc = bacc.Bacc("TRN2", target_bir_lowering=False, debug=False)

    # ---- DRAM I/O ----
    x_d = nc.dram_tensor("x", (C, NS, 1024), F8, kind="ExternalInput")
    w_d = [
        nc.dram_tensor(f"w{i+1}", (C, CFG[i][1] * C), F8, kind="ExternalInput")
        for i in range(4)
    ]
    # bias/scale pack: cols 0-3 = b1..b4; cols 4-7 = 2^-k1..2^-k4;
    # cols 8-71 = ones (zn mm rhs); cols 72-135 = row0-only 1+||c||^2
    # (cn mm rhs); cols 136-167 = e0 (partition-0 ones, cn mm lhsT);
    # col 168 = 16*2^-k4 (z8 scale); col 169 = 16*b4 (z8 bias)
    bp_d = nc.dram_tensor("bp", (C, 170), F32, kind="ExternalInput")
    # fp8 centers: cr8[c, l*64+j] = (-2/16)*centers[j, c*59+l], chunk 59 = 0
    cr_d = nc.dram_tensor("cr", (C, 60 * KCENT), F8, kind="ExternalInput")
    q_d = nc.dram_tensor("q", (NS, KCENT), F32, kind="ExternalOutput")

    with tile.TileContext(nc) as tc:
        with (
            tc.tile_pool(name="consts", bufs=1) as cpool,
            tc.tile_pool(name="xp", bufs=8) as xpool,
            tc.tile_pool(name="hp", bufs=1) as hpool,
            tc.tile_pool(name="ap", bufs=4) as apool,
            tc.tile_pool(name="small", bufs=1) as mpool,
            tc.tile_pool(name="psA", bufs=3, space="PSUM") as psA,
            tc.tile_pool(name="psD", bufs=2, space="PSUM") as psD,
        ):
            wt = [
                cpool.tile([C, CFG[i][1] * C], F8, tag=f"w{i}", name=f"wt{i}")
                for i in range(4)
            ]
            bp = cpool.tile([C, 170], F32, tag="bp")
            cr = cpool.tile([C, 60 * KCENT], F8, tag="cr")

            for _rep in range(n_repeat):
                _body_once(nc, tc, x_d, q_d, w_d, bp_d, cr_d, wt, bp,
                           cr, xpool, hpool, apool, mpool,
                           psA, psD, load_consts=(_rep == 0))

    nc.compile()
    return nc


def _body_once(nc, tc, x_d, q_d, w_d, bp_d, cr_d, wt, bp, cr,
               xpool, hpool, apool, mpool, psA, psD, load_consts=True):
            # ---- Warm-up scratch FIRST so the PE can start ramping while
            # the DMA lead-in runs ----
            if load_consts:
                wsrc = mpool.tile([1, 128], HDT, tag="warm", name="warm")
                nc.gpsimd.memset(wsrc[:], 0.0)

            # ---- Two HWDGE rings. SP ring: w1 taps 0-7, then x samples 0,1,
            # then 2-sample x chunks (smallest-first so conv1 starts early).
            # ACT ring: w1 taps 8-15, bp early; w2-4, cr after the first few
            # x chunks so they don't delay conv1's input stream ----
            w1v = w_d[0].ap().rearrange("p (k o) -> p k o", o=C)
            wt1v = wt[0][:].rearrange("p (k o) -> p k o", o=C)
            if load_consts:
                nc.sync.dma_start(wt1v[:, 0:8, :], w1v[:, 0:8, :])
            x3 = x_d.ap()  # (C, NS, 1024)
            xch = []
            xt = xpool.tile([C, 2 * 1024], F8, tag="x", name="xch0")
            xt3 = xt[:].rearrange("p (a b) -> p a b", a=2)
            nc.sync.dma_start(xt3[:, 0:1, :], x3[:, 0:1, :])
            if load_consts:
                nc.scalar.dma_start(wt1v[:, 8:16, :], w1v[:, 8:16, :])
                nc.scalar.dma_start(bp[:], bp_d.ap())
            nc.sync.dma_start(xt3[:, 1:2, :], x3[:, 1:2, :])
            xch.append(xt)
            for g in range(1, 16):
                t = xpool.tile([C, 2 * 1024], F8, tag="x", name=f"xch{g}")
                src = x3[:, 2 * g : 2 * g + 2, :].rearrange("p a b -> p (a b)")
                nc.sync.dma_start(t[:], src)
                xch.append(t)
                if load_consts and g == 3:
                    for i in range(1, 4):
                        nc.scalar.dma_start(wt[i][:], w_d[i].ap())
                    nc.scalar.dma_start(cr[:], cr_d.ap())

            # ---- PE pre-warm: ramp the p-state (0.65 -> 2.4 GHz needs
            # ~3us of continuous busy) while the first DMAs land ----
            if load_consts:
                wps = psD.tile([C, 128], F32, tag="d", name="warmps")
                for _w in range(N_WARM):
                    nc.tensor.matmul(
                        wps[:], wsrc[:], wsrc[:],
                        start=(_w == 0), stop=(_w == N_WARM - 1)
                    )

            # ---- shared tiles for the conv stack / distance tail ----
            h_tiles = []
            for li, (K, Kp, Lin, Lout, Lrow, G) in enumerate(CFG[:3]):
                hdst = hpool.tile([C, NS * Lrow], F8, tag=f"h{li}")
                if Lrow > Lout:
                    # zero the per-sample pad so the zero tap's in-bounds
                    # read never multiplies NaN garbage
                    padv = hdst[:].rearrange("p (n l) -> p n l", n=NS)
                    nc.gpsimd.memset(padv[:, :, Lout:Lrow], 0.0)
                h_tiles.append(hdst)
            zsq = hpool.tile([C, NS * LFIN], HDT, tag="zsq", name="zsq")
            part = mpool.tile([C, NS], HDT, tag="part", name="part")
            ones = mpool.tile([C, KCENT], HDT, tag="ones", name="ones")
            nc.gpsimd.memset(ones[:], 1.0)
            z8 = hpool.tile([C, 60 * NS], F8, tag="z8", name="z8")
            nc.gpsimd.memset(z8[:, LFIN * NS : 60 * NS], 0.0)
            z84 = z8[:].rearrange("p (l n) -> p l n", n=NS)
            cr3 = cr[:].rearrange("p (l j) -> p l j", j=KCENT)
            dps = [None] * 8

            def conv_block(li, gp, ng, fast):
                """One PSUM block over samples [gp, gp+ng). Pieces of up to
                G samples (<=512 PSUM cols) accumulate into one tile.
                conv1-3 eviction:
                  std:  one ACT affine over the block, then one lrelu STT
                        per piece (DVE piece0, Pool piece1)
                  fast: per-piece ACT + column-split STT on DVE||Pool
                        (lowest latency; used on the tapered tail chain)
                conv4: ACT Identity -> fp8 z8, ACT Square -> bf16 zsq,
                DVE X-reduce -> part (all per block)."""
                K, Kp, Lin, Lout, Lrow, G = CFG[li]
                G = min(G, ng)
                npc = (ng + G - 1) // G
                if li < 3:
                    hdst3 = h_tiles[li][:].rearrange("p (n l) -> p n l", n=NS)
                if li > 0:
                    hsrc3 = h_tiles[li - 1][:].rearrange("p (n l) -> p n l", n=NS)
                ps = psA.tile([C, 1024], F32, tag="ps")
                for pc in range(npc):
                    g0 = gp + pc * G
                    pslice = ps[:, pc * 512 : pc * 512 + G * Lout]
                    for kp in range(0, Kp, 2):
                        lhsT = wt[li][:, kp * C : (kp + 2) * C].rearrange(
                            "p (two o) -> p two o", two=2
                        )
                        if li == 0:
                            xv = xch[g0 // 2][:].rearrange("p (a b) -> p a b", a=2)
                            rhs = xv[
                                :, g0 % 2 : g0 % 2 + 1, kp : kp + 2 * Lout
                            ].rearrange("p n (l two) -> p two n l", two=2)
                        else:
                            rhs = hsrc3[
                                :, g0 : g0 + G, kp : kp + 2 * Lout
                            ].rearrange("p n (l two) -> p two n l", two=2)
                        nc.tensor.matmul(
                            pslice, lhsT, rhs,
                            start=(kp == 0), stop=(kp == Kp - 2),
                            perf_mode=DR,
                        )
                bias = bp[:, li : li + 1]
                scale = bp[:, 4 + li : 5 + li]
                if li < 3:
                    if not fast:
                        psv = (
                            ps[:].rearrange("p (g l) -> p g l", g=npc)[
                                :, :, 0 : G * Lout
                            ]
                            if npc > 1
                            else ps[:, 0 : G * Lout]
                        )
                        A = apool.tile([C, ng * Lout], HDT, tag="A")
                        nc.scalar.activation(
                            A[:], psv, AF.Identity, bias=bias, scale=scale
                        )
                        A3 = A[:].rearrange("p (n l) -> p n l", n=ng)
                        for pc in range(npc):
                            s0, s1 = pc * G, (pc + 1) * G
                            dsl = hdst3[:, gp + s0 : gp + s1, 0:Lout]
                            eng = nc.vector if pc % 2 == 0 else nc.gpsimd
                            eng.scalar_tensor_tensor(
                                dsl, A3[:, s0:s1, :], 0.1, A3[:, s0:s1, :],
                                op0=OP.mult, op1=OP.max,
                            )
                    else:
                        SPL = SPLIT_L[li]
                        for pc in range(npc):
                            g0 = gp + pc * G
                            psh = ps[:, pc * 512 : pc * 512 + G * Lout]
                            A = apool.tile([C, G * Lout], HDT, tag="Af")
                            nc.scalar.activation(
                                A[:], psh, AF.Identity, bias=bias, scale=scale
                            )
                            A3 = A[:].rearrange("p (n l) -> p n l", n=G)
                            dsl = hdst3[:, g0 : g0 + G, 0:Lout]
                            nc.vector.scalar_tensor_tensor(
                                dsl[:, :, 0:SPL], A3[:, :, 0:SPL], 0.1,
                                A3[:, :, 0:SPL], op0=OP.mult, op1=OP.max,
                            )
                            nc.gpsimd.scalar_tensor_tensor(
                                dsl[:, :, SPL:Lout], A3[:, :, SPL:Lout], 0.1,
                                A3[:, :, SPL:Lout], op0=OP.mult, op1=OP.max,
                            )
                else:
                    # conv4 (always one piece: ng*59 <= 472).
                    # std: ACT does both z8 and Square; fast: z8 moves to a
                    # DVE tensor_scalar so ACT's two ops don't serialize on
                    # the tail critical path.
                    psv = ps[:, 0 : ng * Lout]
                    outv = z84[:, 0:LFIN, gp : gp + ng].rearrange("p l n -> p n l")
                    inv = psv.rearrange("p (n l) -> p n l", n=ng)
                    nc.scalar.activation(
                        outv, inv, AF.Identity,
                        bias=bp[:, 169:170], scale=bp[:, 168:169],
                    )
                    zsl = zsq[:, gp * LFIN : (gp + ng) * LFIN]
                    nc.scalar.activation(
                        zsl, psv, AF.Square, bias=bias, scale=scale
                    )
                    with nc.allow_low_precision(
                        "||z||^2 in bf16: ~2^-8 relative on d2's largest "
                        "term, well inside the 2e-2 gate"
                    ):
                        nc.vector.tensor_reduce(
                            part[:, gp : gp + ng],
                            zsl.rearrange("p (n l) -> p n l", n=ng),
                            axis=mybir.AxisListType.X,
                            op=OP.add,
                        )

            def dist_block(didx, g0, rs):
                """d2 for rs samples in one PSUM tile (partition base 0):
                cn (start) -> 30 fp8-DR position-pair chunks -> zn (stop)."""
                dp = psD.tile([rs, KCENT], F32, tag="d")
                dps[didx] = dp
                nc.tensor.matmul(
                    dp[:], bp[:, 136 : 136 + rs], bp[:, 72:136],
                    start=True, stop=False,
                )
                for lp in range(0, 60, 2):
                    lhsT = z84[:, lp : lp + 2, g0 : g0 + rs]
                    nc.tensor.matmul(
                        dp[:], lhsT, cr3[:, lp : lp + 2, :],
                        start=False, stop=False, perf_mode=DR,
                    )
                nc.tensor.matmul(
                    dp[:], part[:, g0 : g0 + rs], ones[:],
                    start=False, stop=True,
                )

            def q_block(didx, g0, rs):
                """q = normalize(1/d2') for rs samples; DMA out per region."""
                dp = dps[didx]
                qn = mpool.tile([rs, KCENT], F32, tag=f"qn{didx}")
                nc.vector.reciprocal(qn[:], dp[:])
                rsum = mpool.tile([rs, 1], F32, tag=f"rs{didx}")
                nc.vector.tensor_reduce(
                    rsum[:], qn[:], axis=mybir.AxisListType.X, op=OP.add
                )
                rr = mpool.tile([rs, 1], F32, tag=f"rr{didx}")
                nc.vector.reciprocal(rr[:], rsum[:])
                nc.vector.tensor_scalar_mul(qn[:], qn[:], rr[:])
                # Late regions DMA on the ACT ring so the SP sequencer's
                # head-of-line DMA wait doesn't stall the final DMA dispatch
                eng = nc.scalar if didx in (2, 3) else nc.sync
                eng.dma_start(q_d.ap()[g0 : g0 + rs, :], qn[:])

            sched = _schedule()
            _check_schedule(sched)
            for blk in sched:
                if blk[0] == "c":
                    conv_block(blk[1], blk[2], blk[3], blk[4])
                elif blk[0] == "d":
                    dist_block(blk[1], blk[2], blk[3])
                else:
                    q_block(blk[1], blk[2], blk[3])


def _get_program(n_repeat=1):
    if n_repeat not in _BUILt:
        _BUILt[n_repeat] = _build_program(n_repeat)
    return _BUILt[n_repeat]


def _to_f8(a):
    """fp32 -> TRN E4M3 (max 240; clip so OCP e4m3fn bit patterns match)."""
    return np.clip(a, -240.0, 240.0).astype(ml_dtypes.float8_e4m3fn)


def _prep_inputs(x, w1, b1, w2, b2, w3, b3, w4, b4, centers):
    """Host-side prep: fp8 quantization, weight transposes, sharding."""
    ws = [w1, w2, w3, w4]
    bs = [b1, b2, b3, b4]

    const_map = {}
    scales = []
    for i, w in enumerate(ws):
        K, Kp = CFG[i][0], CFG[i][1]
        wf = np.asarray(w, np.float32)  # (O, I, K)
        # per-layer power-of-2 scale-up to ~224 max magnitude (e4m3 headroom)
        mx = float(np.abs(wf).max())
        k = int(np.floor(np.log2(224.0 / mx))) if mx > 0 else 0
        scales.append(2.0 ** (-k))
        wq = wf * (2.0 ** k)
        # (O, I, K) -> (I, Kp, O): lhsT tap k = [:, k*128:(k+1)*128]
        wp = np.zeros((C, Kp, C), np.float32)
        wp[:, :K, :] = wq.transpose(1, 2, 0)
        const_map[f"w{i+1}"] = _to_f8(wp.reshape(C, Kp * C))

    cent = np.asarray(centers, np.float32)
    # cr8[c, l*64 + j] = (-2/16) * centers[j, c*59 + l]; position chunk 59
    # is zero (pairs the z8 pad so the DR distance contracts 60 positions).
    # The 1/16 undoes z8's x16 pre-scale (both powers of 2, exact).
    cr8 = np.zeros((C, 60, KCENT), np.float32)
    cr8[:, :LFIN, :] = (
        (-2.0 / 16.0 * cent).reshape(KCENT, C, LFIN).transpose(1, 2, 0)
    )
    const_map["cr"] = _to_f8(cr8.reshape(C, 60 * KCENT))
    cn = 1.0 + (cent.astype(np.float64) ** 2).sum(axis=1)  # (64,)

    bpk = np.zeros((C, 170), np.float32)
    for i, b in enumerate(bs):
        bpk[:, i] = np.asarray(b, np.float32)
        bpk[:, 4 + i] = scales[i]
    bpk[:, 8:72] = 1.0                      # zn mm rhs (ones)
    bpk[0, 72:136] = cn.astype(np.float32)  # cn mm rhs (row 0 only)
    bpk[0, 136:168] = 1.0                   # cn mm lhsT e0 (row 0 only)
    bpk[:, 168] = 16.0 * scales[3]          # z8 scale
    bpk[:, 169] = 16.0 * np.asarray(bs[3], np.float32)  # z8 bias
    const_map["bp"] = bpk

    xf = np.asarray(x, np.float32)
    in_maps = []
    for c in range(N_CORES):
        shard = xf[c * NS : (c + 1) * NS]  # (32, 128, 1024)
        xc = _to_f8(np.ascontiguousarray(shard.transpose(1, 0, 2)))  # (128,32,1024)
        in_maps.append({"x": xc, **const_map})
    return in_maps


def _ensure_devices():
    """Absorb wedged-device attach faults with a tiny op before the real run.

    A previous process can leave a NeuronCore wedged
    (NRT_EXEC_UNIT_UNRECOVERABLE); the first attach after a wedge fails and
    triggers a reset that completes within ~60 s.
    """
    import time

    import jax
    import jax.numpy as jnp

    for attempt in range(3):
        try:
            outs = [jax.device_put(jnp.zeros((8,)), d) + 1.0 for d in jax.devices()]
            jax.block_until_ready(outs)
            return
        except Exception:  # noqa: BLE001 - device fault; wait out the reset
            if attempt == 2:
                raise
            time.sleep(60)


def run(trace=False, **inputs):
    """Run the kernel; returns (q_full, BassKernelResults).

    Retries on device-unrecoverable faults (see _ensure_devices).
    """
    import time

    _ensure_devices()
    nc = _get_program()
    in_maps = _prep_inputs(**inputs)
    last_err = None
    for attempt in range(3):
        try:
            res = bass_utils.run_bass_kernel_spmd(
                nc, in_maps, core_ids=list(range(N_CORES)), trace=trace
            )
            break
        except Exception as e:  # noqa: BLE001 - device fault, wait + retry
            last_err = e
            if "UNAVAILABLE" not in str(e) and "unrecoverable" not in str(e).lower():
                raise
            time.sleep(60)
    else:
        raise last_err
    q = np.concatenate([res.results[c]["q"] for c in range(N_CORES)], axis=0)
    return np.ascontiguousarray(q.astype(np.float32)), res


def kernel(**inputs) -> np.ndarray:
    q, _ = run(trace=False, **inputs)
    return q


# revision 23
# speedup vs baseline: 1.0984x; 1.0123x over previous
"""Trainium2 Bass kernel for nn_DEC_26139170963600 (vq_codebook).

Reference computation:
  4x strided conv1d (stride 2, VALID) with LeakyReLU(0.1) between layers,
  flatten -> soft VQ assignment over 64 centers:
      d2 = ||z||^2 + ||c||^2 - 2 z.c
      q  = (1/(1+d2)) row-normalized            (alpha=1 -> exponent is 1)

Sharding: data-parallel over batch N=256 across 8 cores (32 samples/core).
Weights / centers replicated. No cross-device communication.

Per-core kernel design (fp8 DoubleRow conv stack, v2):
  - x and all conv weights quantized host-side to fp8e4 (TRN E4M3, max 240).
    Weights are pre-scaled by a per-layer power of two (up to ~224 max mag)
    to clear e4m3 subnormals; the inverse scale rides the PSUM eviction.
  - conv layer = K/2 tap-pair matmuls in MatmulPerfMode.DoubleRow, which
    virtualizes the PE to a 256-deep contraction (2 fp8 weights/cell):
        out[o, l] += sum_i W[o,:,k+i]^T . h[:, 2l+k+i],  i in {0,1}
    lhsT = (128, 2, 128) tap-pair slice of the packed weights; rhs is the
    contiguous slice h[k : k+2*Lout] rearranged "(l two) -> two l" (the
    stride-2 conv makes tap pairs adjacent in memory). Odd K padded with a
    zero tap (conv1 15->16, conv3 7->8; h2 rows are stride-248-padded so
    the zero tap's read stays in-bounds).
  - conv1-3 eviction is TWO ops (was 3): ACT A = ps*2^-k + b (PSUM->bf16),
    then LeakyReLU in one scalar_tensor_tensor per half:
        h' = (A * 0.1) max A   (exact lrelu since 0.1 < 1)
    half0 on DVE, half1 on Pool - balances both engines well under the PE
    and removes the old DVE serialization that stalled the PE.
  - conv4 eviction runs ENTIRELY on ACT (DVE only does a small reduce):
    ACT Square(ps*s+b) -> zsq f32; ACT Identity(ps*16s+16b) -> fp8 z8
    (position-major for the DR distance); DVE X-reduce zsq -> part.
  - distance: 4 regions of 8 samples; per region: fp32 cn matmul seeds
    1+||c||^2 (row-0 outer product), 30 fp8-DR position-pair chunks of
    -2 z.c, fp32 part x ones matmul adds ||z||^2 (stop).
  - q = reciprocal(d2') row-normalized on DVE, DMA out per region (fp32).
  - PE pre-warm: dummy matmuls during the DMA lead-in so HAM un-throttles
    (0.65 -> 2.4 GHz) before real conv work; first transfers are split
    (w1 taps 0-7, then single samples) so conv1 starts ~2.8us.
  - CAUTION (probe-verified in a previous session): ACT Prelu would fuse
    the whole lrelu eviction into one op but hard-faults the device when
    >=4 cores run it concurrently. Do not use ACT Prelu / fp16 matmuls.

Measured: see test.py (TimelineSim exec-time metric; baseline was 63209 ns).
"""

import os
import sys

import numpy as np
import ml_dtypes

for _p in ("/opt/trn_rl_repo",):
    if _p not in sys.path and os.path.isdir(_p):
        sys.path.insert(0, _p)

import concourse.bacc as bacc  # noqa: E402
import concourse.mybir as mybir  # noqa: E402
import concourse.tile as tile  # noqa: E402
from concourse import bass_utils  # noqa: E402

F8 = mybir.dt.float8e4
HDT = mybir.dt.bfloat16
F32 = mybir.dt.float32
AF = mybir.ActivationFunctionType
OP = mybir.AluOpType
DR = mybir.MatmulPerfMode.DoubleRow

N_CORES = 8
NS = 32          # samples per core
C = 128          # channels
KCENT = 64       # number of centers
LFIN = 59        # final length
D = C * LFIN     # 7552
NREG = 4         # distance regions (8 samples each)
RS = NS // NREG  # 8 samples per region

# (K_real, K_padded, L_in_row_stride, L_out, L_out_row_stride, G samples/mm)
# conv2's output rows carry 1 pad element (248) so conv3's zero tap 7 reads
# in-bounds; the pad is memset once.
CFG = [
    (15, 16, 1024, 505, 505, 1),
    (12, 12, 505, 247, 248, 2),
    (7, 8, 248, 121, 121, 4),
    (4, 4, 121, 59, 59, 8),
]

N_WARM = 28  # PE pre-warm dummy matmuls
# lrelu STT column split per half: DVE takes the first SPLIT_FRAC of each
# half's columns, Pool the rest (both run concurrently)
SPLIT_L = {0: 278, 1: 136, 2: 67}

_BUILt = {}


def _schedule():
    """Block emission order. Entries:
      ("c", li, g0, ng, fast)  conv block over samples [g0, g0+ng)
      ("d", didx, g0, rs)      distance region
      ("q", didx, g0, rs)      q normalization + DMA out
    Bulk blocks (fast=False) use the throughput eviction; the final chain
    over samples 24-31 is tapered into 2-sample pieces with low-latency
    (fast=True) evictions so the pipeline drain is short."""
    c1 = [("c", 0, 2 * p, 2, False) for p in range(16)]
    c2 = [("c", 1, 4 * p, 4, False) for p in range(7)]
    c3 = [("c", 2, 8 * p, 8, False) for p in range(3)]
    c4 = [("c", 3, 8 * p, 8, False) for p in range(3)]
    return [
        c1[0], c1[1], c1[2], c1[3],
        c1[4], c1[5],
        c2[0],
        c1[6], c1[7],
        c2[1],
        c1[8],
        c3[0],
        c1[9],
        c2[2],
        c1[10],
        c4[0],
        c2[3],
        c1[11],
        ("d", 0, 0, 8),
        c3[1],
        ("q", 0, 0, 8),
        c1[12],
        c2[4],
        c4[1],
        c1[13],
        ("d", 1, 8, 8),
        c2[5],
        c1[14],
        c3[2],
        ("q", 1, 8, 8),
        c1[15],
        c2[6],
        c4[2],
        ("c", 1, 28, 2, True),
        ("d", 2, 16, 8),
        ("c", 1, 30, 2, True),
        ("c", 2, 24, 4, True),
        ("q", 2, 16, 8),
        ("c", 2, 28, 2, True),
        ("c", 2, 30, 2, True),
        ("c", 3, 24, 4, False),
        ("d", 3, 24, 4),
        ("c", 3, 28, 2, True),
        ("c", 3, 30, 2, True),
        ("q", 3, 24, 4),
        ("d", 4, 28, 4),
        ("q", 4, 28, 4),
    ]


def _check_schedule(sched):
    """Topological sanity: every block's producing sample ranges appear
    earlier, and all samples are covered exactly once per stage."""
    done = {li: set() for li in range(4)}
    ddone = {}
    for blk in sched:
        if blk[0] == "c":
            _, li, g0, ng, _fast = blk
            rng = set(range(g0, g0 + ng))
            assert not (rng & done[li]), f"dup {blk}"
            if li > 0:
                assert rng <= done[li - 1], f"{blk} missing producer samples"
            done[li] |= rng
        elif blk[0] == "d":
            _, didx, g0, rs = blk
            rng = set(range(g0, g0 + rs))
            assert rng <= done[3], f"{blk} before conv4"
            ddone[didx] = rng
        else:
            _, didx, g0, rs = blk
            assert ddone.get(didx) == set(range(g0, g0 + rs)), f"{blk} vs dist"
    for li in range(4):
        assert done[li] == set(range(NS)), f"layer {li} incomplete"
    assert set().union(*ddone.values()) == set(range(NS))


def _build_program(n_repeat=1):
    """Build + compile the per-core Bass program (same program on all cores)."""
    nc = bacc.Bacc("TRN2", target_bir_lowering=False, debug=False)

    # ---- DRAM I/O ----
    x_d = nc.dram_tensor("x", (C, NS, 1024), F8, kind="ExternalInput")
    w_d = [
        nc.dram_tensor(f"w{i+1}", (C, CFG[i][1] * C), F8, kind="ExternalInput")
        for i in range(4)
    ]
    # bias/scale pack: cols 0-3 = b1..b4; cols 4-7 = 2^-k1..2^-k4;
    # cols 8-71 = ones (zn mm rhs); cols 72-135 = row0-only 1+||c||^2
    # (cn mm rhs); cols 136-167 = e0 (partition-0 ones, cn mm lhsT);
    # col 168 = 16*2^-k4 (z8 scale); col 169 = 16*b4 (z8 bias)
    bp_d = nc.dram_tensor("bp", (C, 170), F32, kind="ExternalInput")
    # fp8 centers: cr8[c, l*64+j] = (-2/16)*centers[j, c*59+l], chunk 59 = 0
    cr_d = nc.dram_tensor("cr", (C, 60 * KCENT), F8, kind="ExternalInput")
    q_d = nc.dram_tensor("q", (NS, KCENT), F32, kind="ExternalOutput")

    with tile.TileContext(nc) as tc:
        with (
            tc.tile_pool(name="consts", bufs=1) as cpool,
            tc.tile_pool(name="xp", bufs=8) as xpool,
            tc.tile_pool(name="hp", bufs=1) as hpool,
            tc.tile_pool(name="ap", bufs=4) as apool,
            tc.tile_pool(name="small", bufs=1) as mpool,
            tc.tile_pool(name="psA", bufs=3, space="PSUM") as psA,
            tc.tile_pool(name="psD", bufs=2, space="PSUM") as psD,
        ):
            wt = [
                cpool.tile([C, CFG[i][1] * C], F8, tag=f"w{i}", name=f"wt{i}")
                for i in range(4)
            ]
            bp = cpool.tile([C, 170], F32, tag="bp")
            cr = cpool.tile([C, 60 * KCENT], F8, tag="cr")

            for _rep in range(n_repeat):
                _body_once(nc, tc, x_d, q_d, w_d, bp_d, cr_d, wt, bp,
                           cr, xpool, hpool, apool, mpool,
                           psA, psD, load_consts=(_rep == 0))

    nc.compile()
    return nc


def _body_once(nc, tc, x_d, q_d, w_d, bp_d, cr_d, wt, bp, cr,
               xpool, hpool, apool, mpool, psA, psD, load_consts=True):
            # ---- Warm-up scratch FIRST so the PE can start ramping while
            # the DMA lead-in runs ----
            if load_consts:
                wsrc = mpool.tile([1, 128], HDT, tag="warm", name="warm")
                nc.vector.memset(wsrc[:], 0.0)

            # ---- One prioritized DMA stream on the SP ring (the HWDGE is a
            # single shared resource, so ring-splitting just interleaves):
            # w1 taps 0-7, x samples 0,1 (smallest-first so conv1 starts
            # early), then w1 taps 8-15 / bp / the rest of x with w2-w4, cr
            # slotted where their first use allows ----
            w1v = w_d[0].ap().rearrange("p (k o) -> p k o", o=C)
            wt1v = wt[0][:].rearrange("p (k o) -> p k o", o=C)
            if load_consts:
                nc.sync.dma_start(wt1v[:, 0:8, :], w1v[:, 0:8, :])
            x3 = x_d.ap()  # (C, NS, 1024)
            xch = []
            xt = xpool.tile([C, 2 * 1024], F8, tag="x", name="xch0")
            xt3 = xt[:].rearrange("p (a b) -> p a b", a=2)
            nc.sync.dma_start(xt3[:, 0:1, :], x3[:, 0:1, :])
            nc.sync.dma_start(xt3[:, 1:2, :], x3[:, 1:2, :])
            if load_consts:
                nc.sync.dma_start(wt1v[:, 8:16, :], w1v[:, 8:16, :])
                nc.sync.dma_start(bp[:], bp_d.ap())
            xch.append(xt)
            for g in range(1, 16):
                t = xpool.tile([C, 2 * 1024], F8, tag="x", name=f"xch{g}")
                src = x3[:, 2 * g : 2 * g + 2, :].rearrange("p a b -> p (a b)")
                nc.sync.dma_start(t[:], src)
                xch.append(t)
                if load_consts and g == 3:
                    nc.sync.dma_start(wt[1][:], w_d[1].ap())
                if load_consts and g == 5:
                    nc.sync.dma_start(wt[2][:], w_d[2].ap())
                    nc.sync.dma_start(wt[3][:], w_d[3].ap())
                if load_consts and g == 8:
                    nc.sync.dma_start(cr[:], cr_d.ap())

            # ---- PE pre-warm: ramp the p-state (0.65 -> 2.4 GHz needs
            # ~3us of continuous busy) while the first DMAs land ----
            if load_consts:
                wps = psD.tile([C, 128], F32, tag="d", name="warmps")
                for _w in range(N_WARM):
                    nc.tensor.matmul(
                        wps[:], wsrc[:], wsrc[:],
                        start=(_w == 0), stop=(_w == N_WARM - 1)
                    )

            # ---- shared tiles for the conv stack / distance tail ----
            h_tiles = []
            for li, (K, Kp, Lin, Lout, Lrow, G) in enumerate(CFG[:3]):
                hdst = hpool.tile([C, NS * Lrow], F8, tag=f"h{li}")
                if Lrow > Lout:
                    # zero the per-sample pad so the zero tap's in-bounds
                    # read never multiplies NaN garbage
                    padv = hdst[:].rearrange("p (n l) -> p n l", n=NS)
                    nc.gpsimd.memset(padv[:, :, Lout:Lrow], 0.0)
                h_tiles.append(hdst)
            zsq = hpool.tile([C, NS * LFIN], HDT, tag="zsq", name="zsq")
            part = mpool.tile([C, NS], HDT, tag="part", name="part")
            ones = mpool.tile([C, KCENT], HDT, tag="ones", name="ones")
            nc.gpsimd.memset(ones[:], 1.0)
            z8 = hpool.tile([C, 60 * NS], F8, tag="z8", name="z8")
            nc.gpsimd.memset(z8[:, LFIN * NS : 60 * NS], 0.0)
            z84 = z8[:].rearrange("p (l n) -> p l n", n=NS)
            cr3 = cr[:].rearrange("p (l j) -> p l j", j=KCENT)
            dps = [None] * 8

            def conv_block(li, gp, ng, fast):
                """One PSUM block over samples [gp, gp+ng). Pieces of up to
                G samples (<=512 PSUM cols) accumulate into one tile.
                conv1-3 eviction:
                  std:  one ACT affine over the block, then one lrelu STT
                        per piece (DVE piece0, Pool piece1)
                  fast: per-piece ACT + column-split STT on DVE||Pool
                        (lowest latency; used on the tapered tail chain)
                conv4: ACT Identity -> fp8 z8, ACT Square -> bf16 zsq,
                DVE X-reduce -> part (all per block)."""
                K, Kp, Lin, Lout, Lrow, G = CFG[li]
                G = min(G, ng)
                npc = (ng + G - 1) // G
                if li < 3:
                    hdst3 = h_tiles[li][:].rearrange("p (n l) -> p n l", n=NS)
                if li > 0:
                    hsrc3 = h_tiles[li - 1][:].rearrange("p (n l) -> p n l", n=NS)
                ps = psA.tile([C, 1024], F32, tag="ps")
                for pc in range(npc):
                    g0 = gp + pc * G
                    pslice = ps[:, pc * 512 : pc * 512 + G * Lout]
                    for kp in range(0, Kp, 2):
                        lhsT = wt[li][:, kp * C : (kp + 2) * C].rearrange(
                            "p (two o) -> p two o", two=2
                        )
                        if li == 0:
                            xv = xch[g0 // 2][:].rearrange("p (a b) -> p a b", a=2)
                            rhs = xv[
                                :, g0 % 2 : g0 % 2 + 1, kp : kp + 2 * Lout
                            ].rearrange("p n (l two) -> p two n l", two=2)
                        else:
                            rhs = hsrc3[
                                :, g0 : g0 + G, kp : kp + 2 * Lout
                            ].rearrange("p n (l two) -> p two n l", two=2)
                        nc.tensor.matmul(
                            pslice, lhsT, rhs,
                            start=(kp == 0), stop=(kp == Kp - 2),
                            perf_mode=DR,
                        )
                bias = bp[:, li : li + 1]
                scale = bp[:, 4 + li : 5 + li]
                if li < 3:
                    if not fast:
                        psv = (
                            ps[:].rearrange("p (g l) -> p g l", g=npc)[
                                :, :, 0 : G * Lout
                            ]
                            if npc > 1
                            else ps[:, 0 : G * Lout]
                        )
                        A = apool.tile([C, ng * Lout], HDT, tag="A")
                        nc.scalar.activation(
                            A[:], psv, AF.Identity, bias=bias, scale=scale
                        )
                        A3 = A[:].rearrange("p (n l) -> p n l", n=ng)
                        for pc in range(npc):
                            s0, s1 = pc * G, (pc + 1) * G
                            dsl = hdst3[:, gp + s0 : gp + s1, 0:Lout]
                            eng = nc.vector if pc % 2 == 0 else nc.gpsimd
                            eng.scalar_tensor_tensor(
                                dsl, A3[:, s0:s1, :], 0.1, A3[:, s0:s1, :],
                                op0=OP.mult, op1=OP.max,
                            )
                    else:
                        SPL = SPLIT_L[li]
                        for pc in range(npc):
                            g0 = gp + pc * G
                            psh = ps[:, pc * 512 : pc * 512 + G * Lout]
                            A = apool.tile([C, G * Lout], HDT, tag="Af")
                            nc.scalar.activation(
                                A[:], psh, AF.Identity, bias=bias, scale=scale
                            )
                            A3 = A[:].rearrange("p (n l) -> p n l", n=G)
                            dsl = hdst3[:, g0 : g0 + G, 0:Lout]
                            nc.vector.scalar_tensor_tensor(
                                dsl[:, :, 0:SPL], A3[:, :, 0:SPL], 0.1,
                                A3[:, :, 0:SPL], op0=OP.mult, op1=OP.max,
                            )
                            nc.gpsimd.scalar_tensor_tensor(
                                dsl[:, :, SPL:Lout], A3[:, :, SPL:Lout], 0.1,
                                A3[:, :, SPL:Lout], op0=OP.mult, op1=OP.max,
                            )
                else:
                    # conv4 (always one piece: ng*59 <= 472).
                    # std: ACT does both z8 and Square; fast: z8 moves to a
                    # DVE tensor_scalar so ACT's two ops don't serialize on
                    # the tail critical path.
                    psv = ps[:, 0 : ng * Lout]
                    outv = z84[:, 0:LFIN, gp : gp + ng].rearrange("p l n -> p n l")
                    inv = psv.rearrange("p (n l) -> p n l", n=ng)
                    nc.scalar.activation(
                        outv, inv, AF.Identity,
                        bias=bp[:, 169:170], scale=bp[:, 168:169],
                    )
                    zsl = zsq[:, gp * LFIN : (gp + ng) * LFIN]
                    nc.scalar.activation(
                        zsl, psv, AF.Square, bias=bias, scale=scale
                    )
                    with nc.allow_low_precision(
                        "||z||^2 in bf16: ~2^-8 relative on d2's largest "
                        "term, well inside the 2e-2 gate"
                    ):
                        nc.vector.tensor_reduce(
                            part[:, gp : gp + ng],
                            zsl.rearrange("p (n l) -> p n l", n=ng),
                            axis=mybir.AxisListType.X,
                            op=OP.add,
                        )

            def dist_block(didx, g0, rs):
                """d2 for rs samples in one PSUM tile (partition base 0):
                cn (start) -> 30 fp8-DR position-pair chunks -> zn (stop)."""
                dp = psD.tile([rs, KCENT], F32, tag="d")
                dps[didx] = dp
                nc.tensor.matmul(
                    dp[:], bp[:, 136 : 136 + rs], bp[:, 72:136],
                    start=True, stop=False,
                )
                for lp in range(0, 60, 2):
                    lhsT = z84[:, lp : lp + 2, g0 : g0 + rs]
                    nc.tensor.matmul(
                        dp[:], lhsT, cr3[:, lp : lp + 2, :],
                        start=False, stop=False, perf_mode=DR,
                    )
                nc.tensor.matmul(
                    dp[:], part[:, g0 : g0 + rs], ones[:],
                    start=False, stop=True,
                )

            def q_block(didx, g0, rs):
                """q = normalize(1/d2') for rs samples; DMA out per region."""
                dp = dps[didx]
                qn = mpool.tile([rs, KCENT], F32, tag=f"qn{didx}")
                nc.vector.reciprocal(qn[:], dp[:])
                rsum = mpool.tile([rs, 1], F32, tag=f"rs{didx}")
                nc.vector.tensor_reduce(
                    rsum[:], qn[:], axis=mybir.AxisListType.X, op=OP.add
                )
                rr = mpool.tile([rs, 1], F32, tag=f"rr{didx}")
                nc.vector.reciprocal(rr[:], rsum[:])
                nc.vector.tensor_scalar_mul(qn[:], qn[:], rr[:])
                # Spread the q DMAs: sync ring for the early regions, ACT
                # ring for region 3, Pool (SWDGE, no shared-HWDGE slot) for
                # the final region so its dispatch is fully decoupled
                eng = {3: nc.scalar, 4: nc.gpsimd}.get(didx, nc.sync)
                eng.dma_start(q_d.ap()[g0 : g0 + rs, :], qn[:])

            sched = _schedule()
            _check_schedule(sched)
            for blk in sched:
                if blk[0] == "c":
                    conv_block(blk[1], blk[2], blk[3], blk[4])
                elif blk[0] == "d":
                    dist_block(blk[1], blk[2], blk[3])
                else:
                    q_block(blk[1], blk[2], blk[3])


def _get_program(n_repeat=1):
    if n_repeat not in _BUILt:
        _BUILt[n_repeat] = _build_program(n_repeat)
    return _BUILt[n_repeat]


def _to_f8(a):
    """fp32 -> TRN E4M3 (max 240; clip so OCP e4m3fn bit patterns match)."""
    return np.clip(a, -240.0, 240.0).astype(ml_dtypes.float8_e4m3fn)


def _prep_inputs(x, w1, b1, w2, b2, w3, b3, w4, b4, centers):
    """Host-side prep: fp8 quantization, weight transposes, sharding."""
    ws = [w1, w2, w3, w4]
    bs = [b1, b2, b3, b4]

    const_map = {}
    scales = []
    for i, w in enumerate(ws):
        K, Kp = CFG[i][0], CFG[i][1]
        wf = np.asarray(w, np.float32)  # (O, I, K)
        # per-layer power-of-2 scale-up to ~224 max magnitude (e4m3 headroom)
        mx = float(np.abs(wf).max())
        k = int(np.floor(np.log2(224.0 / mx))) if mx > 0 else 0
        scales.append(2.0 ** (-k))
        wq = wf * (2.0 ** k)
        # (O, I, K) -> (I, Kp, O): lhsT tap k = [:, k*128:(k+1)*128]
        wp = np.zeros((C, Kp, C), np.float32)
        wp[:, :K, :] = wq.transpose(1, 2, 0)
        const_map[f"w{i+1}"] = _to_f8(wp.reshape(C, Kp * C))

    cent = np.asarray(centers, np.float32)
    # cr8[c, l*64 + j] = (-2/16) * centers[j, c*59 + l]; position chunk 59
    # is zero (pairs the z8 pad so the DR distance contracts 60 positions).
    # The 1/16 undoes z8's x16 pre-scale (both powers of 2, exact).
    cr8 = np.zeros((C, 60, KCENT), np.float32)
    cr8[:, :LFIN, :] = (
        (-2.0 / 16.0 * cent).reshape(KCENT, C, LFIN).transpose(1, 2, 0)
    )
    const_map["cr"] = _to_f8(cr8.reshape(C, 60 * KCENT))
    cn = 1.0 + (cent.astype(np.float64) ** 2).sum(axis=1)  # (64,)

    bpk = np.zeros((C, 170), np.float32)
    for i, b in enumerate(bs):
        bpk[:, i] = np.asarray(b, np.float32)
        bpk[:, 4 + i] = scales[i]
    bpk[:, 8:72] = 1.0                      # zn mm rhs (ones)
    bpk[0, 72:136] = cn.astype(np.float32)  # cn mm rhs (row 0 only)
    bpk[0, 136:168] = 1.0                   # cn mm lhsT e0 (row 0 only)
    bpk[:, 168] = 16.0 * scales[3]          # z8 scale
    bpk[:, 169] = 16.0 * np.asarray(bs[3], np.float32)  # z8 bias
    const_map["bp"] = bpk

    xf = np.asarray(x, np.float32)
    in_maps = []
    for c in range(N_CORES):
        shard = xf[c * NS : (c + 1) * NS]  # (32, 128, 1024)
        xc = _to_f8(np.ascontiguousarray(shard.transpose(1, 0, 2)))  # (128,32,1024)
        in_maps.append({"x": xc, **const_map})
    return in_maps


def _ensure_devices():
    """Absorb wedged-device attach faults with a tiny op before the real run.

    A previous process can leave a NeuronCore wedged
    (NRT_EXEC_UNIT_UNRECOVERABLE); the first attach after a wedge fails and
    triggers a reset that completes within ~60 s.
    """
    import time

    import jax
    import jax.numpy as jnp

    for attempt in range(3):
        try:
            outs = [jax.device_put(jnp.zeros((8,)), d) + 1.0 for d in jax.devices()]
            jax.block_until_ready(outs)
            return
        except Exception:  # noqa: BLE001 - device fault; wait out the reset
            if attempt == 2:
                raise
            time.sleep(60)


def run(trace=False, **inputs):
    """Run the kernel; returns (q_full, BassKernelResults).

    Retries on device-unrecoverable faults (see _ensure_devices).
    """
    import time

    _ensure_devices()
    nc = _get_program()
    in_maps = _prep_inputs(**inputs)
    last_err = None
    for attempt in range(3):
        try:
            res = bass_utils.run_bass_kernel_spmd(
                nc, in_maps, core_ids=list(range(N_CORES)), trace=trace
            )
            break
        except Exception as e:  # noqa: BLE001 - device fault, wait + retry
            last_err = e
            if "UNAVAILABLE" not in str(e) and "unrecoverable" not in str(e).lower():
                raise
            time.sleep(60)
    else:
        raise last_err
    q = np.concatenate([res.results[c]["q"] for c in range(N_CORES)], axis=0)
    return np.ascontiguousarray(q.astype(np.float32)), res


def kernel(**inputs) -> np.ndarray:
    q, _ = run(trace=False, **inputs)
    return q


# revision 26
# speedup vs baseline: 1.1020x; 1.0032x over previous
"""Trainium2 Bass kernel for nn_DEC_26139170963600 (vq_codebook).

Reference computation:
  4x strided conv1d (stride 2, VALID) with LeakyReLU(0.1) between layers,
  flatten -> soft VQ assignment over 64 centers:
      d2 = ||z||^2 + ||c||^2 - 2 z.c
      q  = (1/(1+d2)) row-normalized            (alpha=1 -> exponent is 1)

Sharding: data-parallel over batch N=256 across 8 cores (32 samples/core).
Weights / centers replicated. No cross-device communication.

Per-core kernel design (fp8 DoubleRow conv stack, v2):
  - x and all conv weights quantized host-side to fp8e4 (TRN E4M3, max 240).
    Weights are pre-scaled by a per-layer power of two (up to ~224 max mag)
    to clear e4m3 subnormals; the inverse scale rides the PSUM eviction.
  - conv layer = K/2 tap-pair matmuls in MatmulPerfMode.DoubleRow, which
    virtualizes the PE to a 256-deep contraction (2 fp8 weights/cell):
        out[o, l] += sum_i W[o,:,k+i]^T . h[:, 2l+k+i],  i in {0,1}
    lhsT = (128, 2, 128) tap-pair slice of the packed weights; rhs is the
    contiguous slice h[k : k+2*Lout] rearranged "(l two) -> two l" (the
    stride-2 conv makes tap pairs adjacent in memory). Odd K padded with a
    zero tap (conv1 15->16, conv3 7->8; h2 rows are stride-248-padded so
    the zero tap's read stays in-bounds).
  - conv1-3 eviction is TWO ops (was 3): ACT A = ps*2^-k + b (PSUM->bf16),
    then LeakyReLU in one scalar_tensor_tensor per half:
        h' = (A * 0.1) max A   (exact lrelu since 0.1 < 1)
    half0 on DVE, half1 on Pool - balances both engines well under the PE
    and removes the old DVE serialization that stalled the PE.
  - conv4 eviction runs ENTIRELY on ACT (DVE only does a small reduce):
    ACT Square(ps*s+b) -> zsq f32; ACT Identity(ps*16s+16b) -> fp8 z8
    (position-major for the DR distance); DVE X-reduce zsq -> part.
  - distance: 4 regions of 8 samples; per region: fp32 cn matmul seeds
    1+||c||^2 (row-0 outer product), 30 fp8-DR position-pair chunks of
    -2 z.c, fp32 part x ones matmul adds ||z||^2 (stop).
  - q = reciprocal(d2') row-normalized on DVE, DMA out per region (fp32).
  - PE pre-warm: dummy matmuls during the DMA lead-in so HAM un-throttles
    (0.65 -> 2.4 GHz) before real conv work; first transfers are split
    (w1 taps 0-7, then single samples) so conv1 starts ~2.8us.
  - CAUTION (probe-verified in a previous session): ACT Prelu would fuse
    the whole lrelu eviction into one op but hard-faults the device when
    >=4 cores run it concurrently. Do not use ACT Prelu / fp16 matmuls.

Measured: see test.py (TimelineSim exec-time metric; baseline was 63209 ns).
"""

import os
import sys

import numpy as np
import ml_dtypes

for _p in ("/opt/trn_rl_repo",):
    if _p not in sys.path and os.path.isdir(_p):
        sys.path.insert(0, _p)

import concourse.bacc as bacc  # noqa: E402
import concourse.mybir as mybir  # noqa: E402
import concourse.tile as tile  # noqa: E402
from concourse import bass_utils  # noqa: E402

F8 = mybir.dt.float8e4
HDT = mybir.dt.bfloat16
F32 = mybir.dt.float32
AF = mybir.ActivationFunctionType
OP = mybir.AluOpType
DR = mybir.MatmulPerfMode.DoubleRow

N_CORES = 8
NS = 32          # samples per core
C = 128          # channels
KCENT = 64       # number of centers
LFIN = 59        # final length
D = C * LFIN     # 7552
NREG = 4         # distance regions (8 samples each)
RS = NS // NREG  # 8 samples per region

# (K_real, K_padded, L_in_row_stride, L_out, L_out_row_stride, G samples/mm)
# conv2's output rows carry 1 pad element (248) so conv3's zero tap 7 reads
# in-bounds; the pad is memset once.
CFG = [
    (15, 16, 1024, 505, 505, 1),
    (12, 12, 505, 247, 248, 2),
    (7, 8, 248, 121, 121, 4),
    (4, 4, 121, 59, 59, 8),
]

N_WARM = 28  # PE pre-warm dummy matmuls
# lrelu STT column split per half: DVE takes the first SPLIT_FRAC of each
# half's columns, Pool the rest (both run concurrently)
SPLIT_L = {0: 278, 1: 136, 2: 67}

_BUILt = {}


def _schedule():
    """Block emission order. Entries:
      ("c", li, g0, ng, fast)  conv block over samples [g0, g0+ng)
      ("d", didx, g0, rs)      distance region
      ("q", didx, g0, rs)      q normalization + DMA out
    Bulk blocks (fast=False) use the throughput eviction; the final chain
    over samples 24-31 is tapered into 2-sample pieces with low-latency
    (fast=True) evictions so the pipeline drain is short."""
    c1 = [("c", 0, 2 * p, 2, False) for p in range(16)]
    c2 = [("c", 1, 4 * p, 4, False) for p in range(7)]
    c3 = [("c", 2, 8 * p, 8, False) for p in range(3)]
    c4 = [("c", 3, 8 * p, 8, False) for p in range(3)]
    return [
        c1[0], c1[1], c1[2], c1[3],
        c1[4], c1[5],
        c2[0],
        c1[6], c1[7],
        c2[1],
        c1[8],
        c3[0],
        c1[9],
        c2[2],
        c1[10],
        c4[0],
        c2[3],
        c1[11],
        ("d", 0, 0, 8),
        c3[1],
        ("q", 0, 0, 8),
        c1[12],
        c2[4],
        c4[1],
        c1[13],
        ("d", 1, 8, 8),
        c2[5],
        c1[14],
        c3[2],
        ("q", 1, 8, 8),
        c1[15],
        c2[6],
        c4[2],
        ("c", 1, 28, 2, True),
        ("d", 2, 16, 8),
        ("c", 1, 30, 2, True),
        ("c", 2, 24, 4, True),
        ("q", 2, 16, 8),
        ("c", 2, 28, 2, True),
        ("c", 2, 30, 2, True),
        ("c", 3, 24, 4, False),
        ("d", 3, 24, 4),
        ("c", 3, 28, 2, True),
        ("c", 3, 30, 2, True),
        ("q", 3, 24, 4),
        ("d", 4, 28, 4),
        ("q", 4, 28, 4),
    ]


def _check_schedule(sched):
    """Topological sanity: every block's producing sample ranges appear
    earlier, and all samples are covered exactly once per stage."""
    done = {li: set() for li in range(4)}
    ddone = {}
    for blk in sched:
        if blk[0] == "c":
            _, li, g0, ng, _fast = blk
            rng = set(range(g0, g0 + ng))
            assert not (rng & done[li]), f"dup {blk}"
            if li > 0:
                assert rng <= done[li - 1], f"{blk} missing producer samples"
            done[li] |= rng
        elif blk[0] == "d":
            _, didx, g0, rs = blk
            rng = set(range(g0, g0 + rs))
            assert rng <= done[3], f"{blk} before conv4"
            ddone[didx] = rng
        else:
            _, didx, g0, rs = blk
            assert ddone.get(didx) == set(range(g0, g0 + rs)), f"{blk} vs dist"
    for li in range(4):
        assert done[li] == set(range(NS)), f"layer {li} incomplete"
    assert set().union(*ddone.values()) == set(range(NS))


def _build_program(n_repeat=1):
    """Build + compile the per-core Bass program (same program on all cores)."""
    nc = bacc.Bacc("TRN2", target_bir_lowering=False, debug=False)

    # ---- DRAM I/O ----
    x_d = nc.dram_tensor("x", (C, NS, 1024), F8, kind="ExternalInput")
    w_d = [
        nc.dram_tensor(f"w{i+1}", (C, CFG[i][1] * C), F8, kind="ExternalInput")
        for i in range(4)
    ]
    # bias/scale pack: cols 0-3 = b1..b4; cols 4-7 = 2^-k1..2^-k4;
    # cols 8-71 = ones (zn mm rhs); cols 72-135 = row0-only 1+||c||^2
    # (cn mm rhs); cols 136-167 = e0 (partition-0 ones, cn mm lhsT);
    # col 168 = 16*2^-k4 (z8 scale); col 169 = 16*b4 (z8 bias)
    bp_d = nc.dram_tensor("bp", (C, 170), F32, kind="ExternalInput")
    # fp8 centers: cr8[c, l*64+j] = (-2/16)*centers[j, c*59+l], chunk 59 = 0
    cr_d = nc.dram_tensor("cr", (C, 60 * KCENT), F8, kind="ExternalInput")
    q_d = nc.dram_tensor("q", (NS, KCENT), F32, kind="ExternalOutput")

    with tile.TileContext(nc) as tc:
        with (
            tc.tile_pool(name="consts", bufs=1) as cpool,
            tc.tile_pool(name="xp", bufs=8) as xpool,
            tc.tile_pool(name="hp", bufs=1) as hpool,
            tc.tile_pool(name="ap", bufs=4) as apool,
            tc.tile_pool(name="small", bufs=1) as mpool,
            tc.tile_pool(name="psA", bufs=3, space="PSUM") as psA,
            tc.tile_pool(name="psD", bufs=2, space="PSUM") as psD,
        ):
            wt = [
                cpool.tile([C, CFG[i][1] * C], F8, tag=f"w{i}", name=f"wt{i}")
                for i in range(4)
            ]
            bp = cpool.tile([C, 170], F32, tag="bp")
            cr = cpool.tile([C, 60 * KCENT], F8, tag="cr")

            for _rep in range(n_repeat):
                _body_once(nc, tc, x_d, q_d, w_d, bp_d, cr_d, wt, bp,
                           cr, xpool, hpool, apool, mpool,
                           psA, psD, load_consts=(_rep == 0))

    nc.compile()
    return nc


def _body_once(nc, tc, x_d, q_d, w_d, bp_d, cr_d, wt, bp, cr,
               xpool, hpool, apool, mpool, psA, psD, load_consts=True):
            # ---- Warm-up scratch FIRST so the PE can start ramping while
            # the DMA lead-in runs ----
            if load_consts:
                wsrc = mpool.tile([1, 128], HDT, tag="warm", name="warm")
                nc.gpsimd.memset(wsrc[:], 0.0)

            # ---- One prioritized DMA stream on the SP ring (the HWDGE is a
            # single shared resource, so ring-splitting just interleaves):
            # w1 taps 0-7, x samples 0,1 (smallest-first so conv1 starts
            # early), then w1 taps 8-15 / bp / the rest of x with w2-w4, cr
            # slotted where their first use allows ----
            w1v = w_d[0].ap().rearrange("p (k o) -> p k o", o=C)
            wt1v = wt[0][:].rearrange("p (k o) -> p k o", o=C)
            if load_consts:
                nc.sync.dma_start(wt1v[:, 0:8, :], w1v[:, 0:8, :])
            x3 = x_d.ap()  # (C, NS, 1024)
            xch = []
            xt = xpool.tile([C, 2 * 1024], F8, tag="x", name="xch0")
            xt3 = xt[:].rearrange("p (a b) -> p a b", a=2)
            nc.sync.dma_start(xt3[:, 0:1, :], x3[:, 0:1, :])
            nc.sync.dma_start(xt3[:, 1:2, :], x3[:, 1:2, :])
            if load_consts:
                nc.sync.dma_start(wt1v[:, 8:16, :], w1v[:, 8:16, :])
                nc.sync.dma_start(bp[:], bp_d.ap())
            xch.append(xt)
            for g in range(1, 16):
                t = xpool.tile([C, 2 * 1024], F8, tag="x", name=f"xch{g}")
                src = x3[:, 2 * g : 2 * g + 2, :].rearrange("p a b -> p (a b)")
                nc.sync.dma_start(t[:], src)
                xch.append(t)
                if load_consts and g == 3:
                    nc.sync.dma_start(wt[1][:], w_d[1].ap())
                if load_consts and g == 5:
                    nc.sync.dma_start(wt[2][:], w_d[2].ap())
                    nc.sync.dma_start(wt[3][:], w_d[3].ap())
                if load_consts and g == 8:
                    nc.sync.dma_start(cr[:], cr_d.ap())

            # ---- PE pre-warm: ramp the p-state (0.65 -> 2.4 GHz needs
            # ~3us of continuous busy) while the first DMAs land ----
            if load_consts:
                wps = psD.tile([C, 128], F32, tag="d", name="warmps")
                for _w in range(N_WARM):
                    nc.tensor.matmul(
                        wps[:], wsrc[:], wsrc[:],
                        start=(_w == 0), stop=(_w == N_WARM - 1)
                    )

            # ---- shared tiles for the conv stack / distance tail ----
            h_tiles = []
            for li, (K, Kp, Lin, Lout, Lrow, G) in enumerate(CFG[:3]):
                hdst = hpool.tile([C, NS * Lrow], F8, tag=f"h{li}")
                if Lrow > Lout:
                    # zero the per-sample pad so the zero tap's in-bounds
                    # read never multiplies NaN garbage
                    padv = hdst[:].rearrange("p (n l) -> p n l", n=NS)
                    nc.gpsimd.memset(padv[:, :, Lout:Lrow], 0.0)
                h_tiles.append(hdst)
            zsq = hpool.tile([C, NS * LFIN], HDT, tag="zsq", name="zsq")
            part = mpool.tile([C, NS], HDT, tag="part", name="part")
            ones = mpool.tile([C, KCENT], HDT, tag="ones", name="ones")
            nc.gpsimd.memset(ones[:], 1.0)
            z8 = hpool.tile([C, 60 * NS], F8, tag="z8", name="z8")
            nc.gpsimd.memset(z8[:, LFIN * NS : 60 * NS], 0.0)
            z84 = z8[:].rearrange("p (l n) -> p l n", n=NS)
            cr3 = cr[:].rearrange("p (l j) -> p l j", j=KCENT)
            dps = [None] * 8

            def conv_block(li, gp, ng, fast):
                """One PSUM block over samples [gp, gp+ng). Pieces of up to
                G samples (<=512 PSUM cols) accumulate into one tile.
                conv1-3 eviction:
                  std:  one ACT affine over the block, then one lrelu STT
                        per piece (DVE piece0, Pool piece1)
                  fast: per-piece ACT + column-split STT on DVE||Pool
                        (lowest latency; used on the tapered tail chain)
                conv4: ACT Identity -> fp8 z8, ACT Square -> bf16 zsq,
                DVE X-reduce -> part (all per block)."""
                K, Kp, Lin, Lout, Lrow, G = CFG[li]
                G = min(G, ng)
                npc = (ng + G - 1) // G
                if li < 3:
                    hdst3 = h_tiles[li][:].rearrange("p (n l) -> p n l", n=NS)
                if li > 0:
                    hsrc3 = h_tiles[li - 1][:].rearrange("p (n l) -> p n l", n=NS)
                ps = psA.tile([C, 1024], F32, tag="ps")
                for pc in range(npc):
                    g0 = gp + pc * G
                    pslice = ps[:, pc * 512 : pc * 512 + G * Lout]
                    for kp in range(0, Kp, 2):
                        lhsT = wt[li][:, kp * C : (kp + 2) * C].rearrange(
                            "p (two o) -> p two o", two=2
                        )
                        if li == 0:
                            xv = xch[g0 // 2][:].rearrange("p (a b) -> p a b", a=2)
                            rhs = xv[
                                :, g0 % 2 : g0 % 2 + 1, kp : kp + 2 * Lout
                            ].rearrange("p n (l two) -> p two n l", two=2)
                        else:
                            rhs = hsrc3[
                                :, g0 : g0 + G, kp : kp + 2 * Lout
                            ].rearrange("p n (l two) -> p two n l", two=2)
                        nc.tensor.matmul(
                            pslice, lhsT, rhs,
                            start=(kp == 0), stop=(kp == Kp - 2),
                            perf_mode=DR,
                        )
                bias = bp[:, li : li + 1]
                scale = bp[:, 4 + li : 5 + li]
                if li < 3:
                    if not fast:
                        psv = (
                            ps[:].rearrange("p (g l) -> p g l", g=npc)[
                                :, :, 0 : G * Lout
                            ]
                            if npc > 1
                            else ps[:, 0 : G * Lout]
                        )
                        A = apool.tile([C, ng * Lout], HDT, tag="A")
                        nc.scalar.activation(
                            A[:], psv, AF.Identity, bias=bias, scale=scale
                        )
                        A3 = A[:].rearrange("p (n l) -> p n l", n=ng)
                        # lrelu STT is DVE-only: the opcode fails the Pool
                        # engine ISA check (neuronxcc NCC_IXCG966)
                        dsl = hdst3[:, gp : gp + ng, 0:Lout]
                        nc.vector.scalar_tensor_tensor(
                            dsl, A3, 0.1, A3, op0=OP.mult, op1=OP.max,
                        )
                    else:
                        for pc in range(npc):
                            g0 = gp + pc * G
                            psh = ps[:, pc * 512 : pc * 512 + G * Lout]
                            A = apool.tile([C, G * Lout], HDT, tag="Af")
                            nc.scalar.activation(
                                A[:], psh, AF.Identity, bias=bias, scale=scale
                            )
                            A3 = A[:].rearrange("p (n l) -> p n l", n=G)
                            dsl = hdst3[:, g0 : g0 + G, 0:Lout]
                            nc.vector.scalar_tensor_tensor(
                                dsl, A3, 0.1, A3, op0=OP.mult, op1=OP.max,
                            )
                else:
                    # conv4 (always one piece: ng*59 <= 472).
                    # std: ACT does both z8 and Square; fast: z8 moves to a
                    # DVE tensor_scalar so ACT's two ops don't serialize on
                    # the tail critical path.
                    psv = ps[:, 0 : ng * Lout]
                    outv = z84[:, 0:LFIN, gp : gp + ng].rearrange("p l n -> p n l")
                    inv = psv.rearrange("p (n l) -> p n l", n=ng)
                    nc.scalar.activation(
                        outv, inv, AF.Identity,
                        bias=bp[:, 169:170], scale=bp[:, 168:169],
                    )
                    zsl = zsq[:, gp * LFIN : (gp + ng) * LFIN]
                    nc.scalar.activation(
                        zsl, psv, AF.Square, bias=bias, scale=scale
                    )
                    with nc.allow_low_precision(
                        "||z||^2 in bf16: ~2^-8 relative on d2's largest "
                        "term, well inside the 2e-2 gate"
                    ):
                        nc.vector.tensor_reduce(
                            part[:, gp : gp + ng],
                            zsl.rearrange("p (n l) -> p n l", n=ng),
                            axis=mybir.AxisListType.X,
                            op=OP.add,
                        )

            def dist_block(didx, g0, rs):
                """d2 for rs samples in one PSUM tile (partition base 0):
                cn (start) -> 30 fp8-DR position-pair chunks -> zn (stop)."""
                dp = psD.tile([rs, KCENT], F32, tag="d")
                dps[didx] = dp
                nc.tensor.matmul(
                    dp[:], bp[:, 136 : 136 + rs], bp[:, 72:136],
                    start=True, stop=False,
                )
                for lp in range(0, 60, 2):
                    lhsT = z84[:, lp : lp + 2, g0 : g0 + rs]
                    nc.tensor.matmul(
                        dp[:], lhsT, cr3[:, lp : lp + 2, :],
                        start=False, stop=False, perf_mode=DR,
                    )
                nc.tensor.matmul(
                    dp[:], part[:, g0 : g0 + rs], ones[:],
                    start=False, stop=True,
                )

            def q_block(didx, g0, rs):
                """q = normalize(1/d2') for rs samples; DMA out per region."""
                dp = dps[didx]
                qn = mpool.tile([rs, KCENT], F32, tag=f"qn{didx}")
                nc.vector.reciprocal(qn[:], dp[:])
                rsum = mpool.tile([rs, 1], F32, tag=f"rs{didx}")
                nc.vector.tensor_reduce(
                    rsum[:], qn[:], axis=mybir.AxisListType.X, op=OP.add
                )
                nc.vector.tensor_scalar(
                    qn[:], qn[:], rsum[:], None, op0=OP.divide
                )
                # Spread the q DMAs: region 3 on the ACT ring so the SP
                # sequencer's head-of-line DMA wait can't stall the final
                # region's dispatch
                eng = {3: nc.scalar}.get(didx, nc.sync)
                eng.dma_start(q_d.ap()[g0 : g0 + rs, :], qn[:])

            sched = _schedule()
            _check_schedule(sched)
            for blk in sched:
                if blk[0] == "c":
                    conv_block(blk[1], blk[2], blk[3], blk[4])
                elif blk[0] == "d":
                    dist_block(blk[1], blk[2], blk[3])
                else:
                    q_block(blk[1], blk[2], blk[3])


def _get_program(n_repeat=1):
    if n_repeat not in _BUILt:
        _BUILt[n_repeat] = _build_program(n_repeat)
    return _BUILt[n_repeat]


def _to_f8(a):
    """fp32 -> TRN E4M3 (max 240; clip so OCP e4m3fn bit patterns match)."""
    return np.clip(a, -240.0, 240.0).astype(ml_dtypes.float8_e4m3fn)


def _prep_inputs(x, w1, b1, w2, b2, w3, b3, w4, b4, centers):
    """Host-side prep: fp8 quantization, weight transposes, sharding."""
    ws = [w1, w2, w3, w4]
    bs = [b1, b2, b3, b4]

    const_map = {}
    scales = []
    for i, w in enumerate(ws):
        K, Kp = CFG[i][0], CFG[i][1]
        wf = np.asarray(w, np.float32)  # (O, I, K)
        # per-layer power-of-2 scale-up to ~224 max magnitude (e4m3 headroom)
        mx = float(np.abs(wf).max())
        k = int(np.floor(np.log2(224.0 / mx))) if mx > 0 else 0
        scales.append(2.0 ** (-k))
        wq = wf * (2.0 ** k)
        # (O, I, K) -> (I, Kp, O): lhsT tap k = [:, k*128:(k+1)*128]
        wp = np.zeros((C, Kp, C), np.float32)
        wp[:, :K, :] = wq.transpose(1, 2, 0)
        const_map[f"w{i+1}"] = _to_f8(wp.reshape(C, Kp * C))

    cent = np.asarray(centers, np.float32)
    # cr8[c, l*64 + j] = (-2/16) * centers[j, c*59 + l]; position chunk 59
    # is zero (pairs the z8 pad so the DR distance contracts 60 positions).
    # The 1/16 undoes z8's x16 pre-scale (both powers of 2, exact).
    cr8 = np.zeros((C, 60, KCENT), np.float32)
    cr8[:, :LFIN, :] = (
        (-2.0 / 16.0 * cent).reshape(KCENT, C, LFIN).transpose(1, 2, 0)
    )
    const_map["cr"] = _to_f8(cr8.reshape(C, 60 * KCENT))
    cn = 1.0 + (cent.astype(np.float64) ** 2).sum(axis=1)  # (64,)

    bpk = np.zeros((C, 170), np.float32)
    for i, b in enumerate(bs):
        bpk[:, i] = np.asarray(b, np.float32)
        bpk[:, 4 + i] = scales[i]
    bpk[:, 8:72] = 1.0                      # zn mm rhs (ones)
    bpk[0, 72:136] = cn.astype(np.float32)  # cn mm rhs (row 0 only)
    bpk[0, 136:168] = 1.0                   # cn mm lhsT e0 (row 0 only)
    bpk[:, 168] = 16.0 * scales[3]          # z8 scale
    bpk[:, 169] = 16.0 * np.asarray(bs[3], np.float32)  # z8 bias
    const_map["bp"] = bpk

    xf = np.asarray(x, np.float32)
    in_maps = []
    for c in range(N_CORES):
        shard = xf[c * NS : (c + 1) * NS]  # (32, 128, 1024)
        xc = _to_f8(np.ascontiguousarray(shard.transpose(1, 0, 2)))  # (128,32,1024)
        in_maps.append({"x": xc, **const_map})
    return in_maps


def _ensure_devices():
    """Absorb wedged-device attach faults with a tiny op before the real run.

    A previous process can leave a NeuronCore wedged
    (NRT_EXEC_UNIT_UNRECOVERABLE); the first attach after a wedge fails and
    triggers a reset that completes within ~60 s.
    """
    import time

    import jax
    import jax.numpy as jnp

    for attempt in range(3):
        try:
            outs = [jax.device_put(jnp.zeros((8,)), d) + 1.0 for d in jax.devices()]
            jax.block_until_ready(outs)
            return
        except Exception:  # noqa: BLE001 - device fault; wait out the reset
            if attempt == 2:
                raise
            time.sleep(60)


def run(trace=False, **inputs):
    """Run the kernel; returns (q_full, BassKernelResults).

    Retries on device-unrecoverable faults (see _ensure_devices).
    """
    import time

    _ensure_devices()
    nc = _get_program()
    in_maps = _prep_inputs(**inputs)
    last_err = None
    for attempt in range(3):
        try:
            res = bass_utils.run_bass_kernel_spmd(
                nc, in_maps, core_ids=list(range(N_CORES)), trace=trace
            )
            break
        except Exception as e:  # noqa: BLE001 - device fault, wait + retry
            last_err = e
            if "UNAVAILABLE" not in str(e) and "unrecoverable" not in str(e).lower():
                raise
            time.sleep(60)
    else:
        raise last_err
    q = np.concatenate([res.results[c]["q"] for c in range(N_CORES)], axis=0)
    return np.ascontiguousarray(q.astype(np.float32)), res


def kernel(**inputs) -> np.ndarray:
    q, _ = run(trace=False, **inputs)
    return q
